# revision 39
# baseline (speedup 1.0000x reference)
"""Trainium2 Bass kernel for nn_DetectionOutput (decode + per-class NMS + top-k).

Sharding: 8 cores = 4 images x 2 class-halves. Core c handles image b=c//2,
classes cm in [40h, 40h+40) where h=c%2 (cm = class-1, i.e. background dropped).

Algorithm (exact, certified): with uniform scores the per-image top-100 cutoff
is ~0.999 while the 25th-best score of any class is <=0.993, so only the top
KN=24 boxes per class can reach the output. Greedy-NMS keep of a sorted prefix
depends only on that prefix, so each core:
  1. 4 max8 rounds per class -> top-24 scores+indices, rank-24 cert value
     (DVE max8/find_index8/match_replace; ties resolve index-ascending,
     matching jnp stable argsort)
  2. per-round GPSIMD indirect_copy gathers of roi+delta planes for the 8
     freshly selected boxes, hidden under the next round's DVE work
  3. decode + clip boxes                            (DVE + one ACT exp)
  4. 24x24 IoU>0.7 strict-lower suppression matrix  (DVE broadcast ops)
  5. NMS keep = fixpoint of k <- valid & ~(M k), 3 iterations (covers
     suppression-chain depth 2; measured depth on this data is 1), with the
     k3==k2 residual shipped to the host
  6. emits kept-masked scores, boxes, cert + convergence meta
Host merges the two half-image candidate sets per image with the reference
tie-break (score desc, candidate index asc), certifies the prefix bound
(tau_100 > max cert, margin ~0.007 on uniform scores) and the fixpoint
residual; an exact numpy fallback runs if either check ever fails, so the
kernel is exact for any input.
"""
import sys

sys.path.insert(0, "/opt/trn_rl_repo")

import numpy as np
import concourse.bass as bass
import concourse.mybir as mybir
from concourse.tile import TileContext

F32 = mybir.dt.float32
U32 = mybir.dt.uint32
U16 = mybir.dt.uint16
Alu = mybir.AluOpType
Act = mybir.ActivationFunctionType

B, N, C = 4, 2000, 81
Cm = C - 1
P = 40            # class-problems per core
NPAD = 2048
K = 32            # scores extracted per class (4 max8 rounds)
KN = 24           # NMS prefix = gathered slots; cert score = rank KN
NROUND = K // 8       # top-K extracted; prefix = first KN, cert = vals[:, KN]
NT = 5            # gather tile-groups of 8 problems
GW = NT * KN      # gather scratch cols
MAX_DET = 100
SCORE_THR = 0.01
NMS_THR = 0.7
MAX_LOG_WH = float(np.log(1000.0 / 16.0))
NEG = -1.0e30

_CACHED_NC = None


def _patch_tile_tail_drain():
    """This walrus build rejects CTRL instructions carrying >2 sync waits
    (NCC_INLA001 'Too many sync wait commands' on the Tile tail drain).
    Emit sync-engine NOPs before the drain and spread the waits out, one
    per instruction."""
    import concourse.tile as tile_mod
    from concourse.vector_clock import ScopedClock

    if getattr(tile_mod.TileContext, "_tail_drain_patched", False):
        return

    def _drain_and_barrier(self, tick_clock, wait_clock):
        nc = self.nc
        nops = [nc.sync.nop(nofuse=True) for _ in range(30)]
        drain_inst = nc.sync.drain()
        wait_clock.add_sem_waits(
            drain_inst.ins, ScopedClock({None: tick_clock.global_clock})
        )
        waits = list(drain_inst.ins.sync_info.on_wait or [])
        if len(waits) > 1:
            assert len(waits) <= len(nops) + 1
            drain_inst.ins.sync_info.on_wait = [waits[0]]
            for w, nop in zip(waits[1:], nops):
                nop.ins.sync_info = mybir.SyncInfo(on_wait=[w], on_update=[])
        nc.all_engine_barrier()
        assert self.sems is not None
        popped = nc._tile_sem_poison_stack.pop()
        assert popped is self._sem_poison
        nc.clear_and_free_semaphores(list(self.sems.allocated().values()))
        nc.all_engine_barrier()

    tile_mod.TileContext._drain_and_barrier = _drain_and_barrier
    tile_mod.TileContext._tail_drain_patched = True


def _split_sync_waits(nc, max_waits=1):
    """Walrus codegen in this container rejects instructions carrying more
    than 1-2 sync waits (class-dependent). Cap every instruction at
    `max_waits` by hoisting the excess onto same-engine NoOps inserted
    immediately before it (engine blocks at the same program point, so
    semantics and Tile's schedule-order guarantees are preserved)."""
    import bass_rust
    ctr = 0
    for f in nc.m.functions:
        for bb in f.blocks:
            ins_list = bb.instructions
            new = []
            for inst in ins_list:
                si = inst.sync_info
                waits = list(si.on_wait) if si and si.on_wait else []
                if len(waits) > max_waits:
                    for w in waits[max_waits:]:
                        ctr += 1
                        nop = bass_rust.InstNoOp(name=f"WSPLIT-{ctr}")
                        nop.engine = inst.engine
                        nop.sync_info = mybir.SyncInfo(on_wait=[w], on_update=[])
                        new.append(nop)
                    inst.sync_info = mybir.SyncInfo(
                        on_wait=waits[:max_waits],
                        on_update=list(si.on_update) if si.on_update else [])
                new.append(inst)
            ins_list[:] = new
    return ctr


def build_program(split_waits=True):
    _patch_tile_tail_drain()
    nc = bass.Bass()
    sc_in = nc.dram_tensor("sc_in", [P, NPAD], F32, kind="ExternalInput")
    planes_in = nc.dram_tensor("planes_in", [NT, 128, N], F32, kind="ExternalInput")
    clip_in = nc.dram_tensor("clip_in", [P, 2], F32, kind="ExternalInput")  # [Wm, Hm]
    oboxes = nc.dram_tensor("oboxes", [4, P, KN], F32, kind="ExternalOutput")
    omask = nc.dram_tensor("omask", [P, KN], F32, kind="ExternalOutput")
    ometa = nc.dram_tensor("ometa", [P, 40], F32, kind="ExternalOutput")
    tri_in = nc.dram_tensor("tri_in", [KN * KN], F32, kind="ExternalInput")
    oidx = nc.dram_tensor("oidx", [P, KN], U32, kind="ExternalOutput")
    scr_gat = nc.dram_tensor("scr_gat", [128 * GW], F32)

    with TileContext(nc) as tc:
        with (
            tc.tile_pool(name="pool", bufs=1) as pool,
            tc.tile_pool(name="plpool", bufs=1) as plpool,
        ):
            sc = pool.tile([P, NPAD], F32)
            nc.sync.dma_start(sc[:], sc_in[:])
            clip = pool.tile([P, 2], F32)
            nc.sync.dma_start(clip[:], clip_in[:])

            # plane tiles up-front; their DMAs overlap the topk rounds
            pls = []
            for t in range(NT):
                pl = plpool.tile([128, N], F32, tag=f"pl{t}", name=f"pl{t}")
                nc.sync.dma_start(pl[:], planes_in[t])
                pls.append(pl)

            # ---- stage 1+2: topk rounds with per-round gather ----
            # idxr row p holds round r's 8 indices at cols 16r..16r+8 (u16);
            # iwt_rt[16g+j, 0] = idxr[8t+g, 16r+j] so indirect_copy's "(s p)"
            # unwrap yields slots 8r..8r+7 in order. Gathers for round r run
            # on GPSIMD under round r+1's DVE work.
            vals = pool.tile([P, NROUND * 8], F32)
            idx = pool.tile([P, NROUND * 8], U32)
            idxr = pool.tile([P, 16 * NROUND], U16)
            nc.vector.memset(idxr[:], 0)
            gat = pool.tile([128, NT * KN], F32)
            irap = idxr[:]
            pk = []
            for k in range(8):
                pkt = pool.tile([P, KN], F32, tag=f"pk{k}", name=f"pk{k}")
                pk.append(pkt)
            for r in range(NROUND):
                s8 = slice(r * 8, (r + 1) * 8)
                nc.vector.max(vals[:, s8], sc[:])
                if r * 8 >= KN:
                    continue    # cert-only round: no indices, no gather
                nc.vector.max_index(idx[:, s8], vals[:, s8], sc[:])
                if r + 1 < NROUND:
                    nc.vector.match_replace(sc[:], vals[:, s8], sc[:], NEG)
                nc.vector.tensor_copy(idxr[:, 16 * r:16 * r + 8], idx[:, s8])
                for t in range(NT):
                    iwt = plpool.tile([128, 1], U16, tag=f"iw{t}_{r}",
                                      name=f"iw{t}_{r}")
                    nc.sync.dma_start(
                        iwt[:], bass.AP(irap.tensor,
                                        irap.offset + 8 * t * 16 * NROUND + 16 * r,
                                        [[16 * NROUND, 8], [1, 16], [1, 1]]))
                    nc.gpsimd.indirect_copy(
                        gat[:, t * KN + 8 * r:t * KN + 8 * r + 8].rearrange(
                            "p (i one) -> p i one", one=1),
                        pls[t][:], iwt[:], True)
                # bounce this round's gathered columns to DRAM for un-interleave
                gap = gat[:]
                nc.sync.dma_start(
                    bass.AP(scr_gat, 8 * r, [[GW, 128], [KN, NT], [1, 8]]),
                    bass.AP(gap.tensor, gap.offset + 8 * r,
                            [[GW, 128], [KN, NT], [1, 8]]))
                if r == 1:
                    for k in range(8):
                        nc.sync.dma_start(
                            pk[k][:, 0:16],
                            bass.AP(scr_gat, k * GW,
                                    [[KN, NT], [16 * GW, 8], [1, 16]]))
                elif r == 2:
                    for k in range(8):
                        nc.sync.dma_start(
                            pk[k][:, 16:KN],
                            bass.AP(scr_gat, k * GW + 16,
                                    [[KN, NT], [16 * GW, 8], [1, 8]]))
            nc.sync.dma_start(ometa[:, 0:NROUND * 8], vals[:])
            nc.sync.dma_start(oidx[:], idx[:, :KN])
            x1r, y1r, x2r, y2r, dx, dy, dw, dh = pk

            _tagn = [0]

            def tile():
                _tagn[0] += 1
                return pool.tile([P, KN], F32, tag=f"dec{_tagn[0]}", name=f"dec{_tagn[0]}")

            V = nc.vector
            # ---- stage 3: decode (mirrors reference fp op order), emitted in
            # two slot waves so slots 0:16 decode under the cert round while
            # round 2's gather is still in flight ----
            w = pool.tile([P, KN], F32, tag="w")
            h = pool.tile([P, KN], F32, tag="h")
            cx, cy, pcx, pcy, pw, ph, area = (tile() for _ in range(7))
            ew = pool.tile([P, 2 * KN], F32, tag="ew")
            aw = pool.tile([P, KN], F32, tag="aw")
            x1o, y1o, x2o, y2o = x1r, y1r, x2r, y2r

            def decode_wave(sl):
                s2 = slice(KN + sl.start, KN + sl.stop)
                V.tensor_tensor(w[:, sl], x2r[:, sl], x1r[:, sl], Alu.subtract)
                V.tensor_scalar_add(w[:, sl], w[:, sl], 1.0)
                V.tensor_tensor(h[:, sl], y2r[:, sl], y1r[:, sl], Alu.subtract)
                V.tensor_scalar_add(h[:, sl], h[:, sl], 1.0)
                V.tensor_scalar_mul(cx[:, sl], w[:, sl], 0.5)
                V.tensor_tensor(cx[:, sl], x1r[:, sl], cx[:, sl], Alu.add)
                V.tensor_scalar_mul(cy[:, sl], h[:, sl], 0.5)
                V.tensor_tensor(cy[:, sl], y1r[:, sl], cy[:, sl], Alu.add)
                V.tensor_scalar_mul(pcx[:, sl], dx[:, sl], 0.1)
                V.tensor_tensor(pcx[:, sl], pcx[:, sl], w[:, sl], Alu.mult)
                V.tensor_tensor(pcx[:, sl], pcx[:, sl], cx[:, sl], Alu.add)
                V.tensor_scalar_mul(pcy[:, sl], dy[:, sl], 0.1)
                V.tensor_tensor(pcy[:, sl], pcy[:, sl], h[:, sl], Alu.mult)
                V.tensor_tensor(pcy[:, sl], pcy[:, sl], cy[:, sl], Alu.add)
                V.tensor_scalar(ew[:, sl], dw[:, sl], 0.2, MAX_LOG_WH,
                                Alu.mult, Alu.min)
                V.tensor_scalar(ew[:, s2], dh[:, sl], 0.2, MAX_LOG_WH,
                                Alu.mult, Alu.min)
                nc.scalar.activation(ew[:, sl], ew[:, sl], Act.Exp)
                nc.scalar.activation(ew[:, s2], ew[:, s2], Act.Exp)
                V.tensor_tensor(pw[:, sl], ew[:, sl], w[:, sl], Alu.mult)
                V.tensor_tensor(ph[:, sl], ew[:, s2], h[:, sl], Alu.mult)
                V.tensor_scalar_mul(pw[:, sl], pw[:, sl], 0.5)
                V.tensor_tensor(x1o[:, sl], pcx[:, sl], pw[:, sl], Alu.subtract)
                V.tensor_tensor(x2o[:, sl], pcx[:, sl], pw[:, sl], Alu.add)
                V.tensor_scalar_add(x2o[:, sl], x2o[:, sl], -1.0)
                V.tensor_scalar_mul(ph[:, sl], ph[:, sl], 0.5)
                V.tensor_tensor(y1o[:, sl], pcy[:, sl], ph[:, sl], Alu.subtract)
                V.tensor_tensor(y2o[:, sl], pcy[:, sl], ph[:, sl], Alu.add)
                V.tensor_scalar_add(y2o[:, sl], y2o[:, sl], -1.0)
                for tl, cc in ((x1o, 0), (x2o, 0), (y1o, 1), (y2o, 1)):
                    V.tensor_scalar(tl[:, sl], tl[:, sl], 0.0,
                                    clip[:, cc:cc + 1], Alu.max, Alu.min)
                V.tensor_tensor(aw[:, sl], x2o[:, sl], x1o[:, sl], Alu.subtract)
                V.tensor_scalar_add(aw[:, sl], aw[:, sl], 1.0)
                V.tensor_tensor(area[:, sl], y2o[:, sl], y1o[:, sl], Alu.subtract)
                V.tensor_scalar_add(area[:, sl], area[:, sl], 1.0)
                V.tensor_tensor(area[:, sl], area[:, sl], aw[:, sl], Alu.mult)

            decode_wave(slice(0, 16))
            decode_wave(slice(16, KN))

            # ---- stage 4: suppression matrix M[p, i, j] = IoU(i,j) > thr ----
            def iview(t):
                return t[:, :, None].broadcast_to([P, KN, KN])

            def jview(t):
                return t[:, None, :].broadcast_to([P, KN, KN])

            def big():
                _tagn[0] += 1
                tl = pool.tile([P, KN * KN], F32, tag=f"big{_tagn[0]}", name=f"big{_tagn[0]}")
                return tl, tl[:].rearrange("p (i j) -> p i j", i=KN)

            M, Mv = big()
            w_t, w_v = big()
            xx1, xx1v = big()
            V.tensor_tensor(xx1v, iview(x1o), jview(x1o), Alu.max)
            V.tensor_tensor(Mv, iview(x2o), jview(x2o), Alu.min)
            V.tensor_tensor(w_v, Mv, xx1v, Alu.subtract)
            V.tensor_scalar(w_t[:], w_t[:], 1.0, 0.0, Alu.add, Alu.max)
            V.tensor_tensor(xx1v, iview(y1o), jview(y1o), Alu.max)
            V.tensor_tensor(Mv, iview(y2o), jview(y2o), Alu.min)
            V.tensor_tensor(xx1v, Mv, xx1v, Alu.subtract)
            V.tensor_scalar(xx1[:], xx1[:], 1.0, 0.0, Alu.add, Alu.max)
            inter = w_t
            V.tensor_tensor(inter[:], w_t[:], xx1[:], Alu.mult)   # inter
            V.tensor_tensor(Mv, jview(area), iview(area), Alu.add)
            V.tensor_tensor(M[:], M[:], inter[:], Alu.subtract)    # union
            V.tensor_scalar_mul(M[:], M[:], NMS_THR)
            V.tensor_tensor(M[:], inter[:], M[:], Alu.is_gt)       # M flags

            # ---- stage 5: NMS via fixpoint iteration ----
            # M[p, a, b] = (IoU(a,b) > thr) & (b < a): k_{t+1}[a] =
            # valid[a] & ~any_b(M[a,b] & k_t[b]). The greedy keep set is the
            # unique fixpoint; 2 iterations cover suppression-chain depth 1
            # (measured depth on this data: 1) and the k4==k3 convergence
            # residual ships to the host, which falls back to an exact
            # reference recompute if it is ever nonzero.
            TRI = pool.tile([P, KN * KN], F32, tag="TRI")
            nc.sync.dma_start(TRI[:], bass.AP(tri_in, 0, [[0, P], [1, KN * KN]]))
            V.tensor_tensor(M[:], M[:], TRI[:], Alu.mult)
            valid = pool.tile([P, KN], F32, tag="valid")
            V.tensor_single_scalar(valid[:], vals[:, :KN], SCORE_THR, Alu.is_gt)
            ka = pool.tile([P, KN], F32, tag="ka")
            kb = pool.tile([P, KN], F32, tag="kb")
            supp = pool.tile([P, KN], F32, tag="supp")
            TMP, TMPv = big()
            k_prev, k_cur = None, valid
            for it in range(2):
                k_next = ka if it % 2 == 0 else kb
                V.tensor_tensor(
                    TMPv, Mv,
                    k_cur[:, None, :].broadcast_to([P, KN, KN]), Alu.mult)
                V.tensor_reduce(supp[:], TMPv, mybir.AxisListType.X, Alu.max)
                V.scalar_tensor_tensor(k_next[:], supp[:], 0.0, valid[:],
                                       Alu.is_equal, Alu.mult)
                k_prev, k_cur = k_cur, k_next
            dtile = pool.tile([P, KN], F32, tag="dtile")
            V.tensor_tensor(dtile[:], k_cur[:], k_prev[:], Alu.not_equal)
            dsum = pool.tile([P, 1], F32, tag="dsum")
            V.tensor_reduce(dsum[:], dtile[:], mybir.AxisListType.X, Alu.add)
            nc.sync.dma_start(ometa[:, 32:33], dsum[:])

            # ---- stage 6: masked scores + outputs ----
            good = k_cur
            pen = pool.tile([P, KN], F32, tag="pen")
            V.tensor_scalar(pen[:], good[:], -NEG, NEG, Alu.mult, Alu.add)  # 0 kept, NEG else
            V.tensor_tensor(good[:], good[:], vals[:, :KN], Alu.mult)
            V.tensor_tensor(good[:], good[:], pen[:], Alu.add)
            nc.sync.dma_start(omask[:], good[:])
            for kk, tl in enumerate((x1o, y1o, x2o, y2o)):
                nc.sync.dma_start(oboxes[kk], tl[:])
    if split_waits:
        _split_sync_waits(nc)
    return nc


# ---------------------------------------------------------------- host side

def _prep_core(all_rois, all_box_deltas, all_cls_scores, im_info, core):
    b, h = core // 2, core % 2
    sc = np.full((P, NPAD), NEG, np.float32)
    sc[:, :N] = all_cls_scores.reshape(B, N, C)[b, :, 1 + 40 * h:41 + 40 * h].T
    planes = np.zeros((NT, 8, 16, N), np.float32)
    planes[:, :, 0:4, :] = all_rois[b].T[None, None]
    dsl = all_box_deltas.reshape(B, N, C * 4)[b][:, 4 + 160 * h:164 + 160 * h]
    planes[:, :, 4:8, :] = dsl.T.reshape(NT, 8, 4, N)
    clip = np.empty((P, 2), np.float32)
    clip[:, 0] = np.float32(im_info[b, 1]) - np.float32(1.0)
    clip[:, 1] = np.float32(im_info[b, 0]) - np.float32(1.0)
    a = np.arange(KN)
    tri = (a[None, :] < a[:, None]).astype(np.float32).reshape(-1)
    return {"sc_in": np.ascontiguousarray(sc),
            "planes_in": np.ascontiguousarray(planes.reshape(NT, 128, N)),
            "clip_in": clip, "tri_in": tri}


def _merge(results):
    out_boxes = np.zeros((B, MAX_DET, 4), np.float32)
    out_scores = np.zeros((B, MAX_DET), np.float32)
    out_classes = np.zeros((B, MAX_DET), np.int32)
    ok_all = True
    for b in range(B):
        r0, r1 = results[2 * b], results[2 * b + 1]
        masked = np.concatenate([r0["omask"], r1["omask"]], axis=0)      # [80, K]
        boxes = np.concatenate([r0["oboxes"], r1["oboxes"]], axis=1)     # [4, 80, K]
        cert = max(float(r0["ometa"][:, KN].max()), float(r1["ometa"][:, KN].max()))
        if float(r0["ometa"][:, 32].sum()) != 0.0 or float(r1["ometa"][:, 32].sum()) != 0.0:
            ok_all = False
            break
        flat = masked.reshape(-1)
        order = np.argsort(-flat, kind="stable")[:MAX_DET]
        ssel = flat[order]
        if not (ssel[-1] > cert and ssel[-1] > -1.0e29):
            ok_all = False
            break
        cm_sel = order // KN
        out_scores[b] = ssel
        out_classes[b] = (cm_sel + 1).astype(np.int32)
        bt = boxes.reshape(4, -1)
        out_boxes[b] = bt[:, order].T
    if not ok_all:
        return None
    out_batch = np.repeat(np.arange(B, dtype=np.int32), MAX_DET)
    return (out_boxes.reshape(-1, 4), out_classes.reshape(-1),
            out_scores.reshape(-1), out_batch)


def kernel(all_rois, all_box_deltas, all_cls_scores, im_info, _sim=False, _trace=False):
    global _CACHED_NC
    all_rois = np.asarray(all_rois, np.float32)
    all_box_deltas = np.asarray(all_box_deltas, np.float32)
    all_cls_scores = np.asarray(all_cls_scores, np.float32)
    im_info = np.asarray(im_info, np.float32)

    if _sim:
        nc = build_program(split_waits=False)
    else:
        if _CACHED_NC is None:
            _CACHED_NC = build_program()
        nc = _CACHED_NC
    in_maps = [_prep_core(all_rois, all_box_deltas, all_cls_scores, im_info, c)
               for c in range(8)]

    if _sim:
        from concourse import bass_interp
        results = []
        for c in range(8):
            sim = bass_interp.CoreSim(nc)
            for k, v in in_maps[c].items():
                sim.tensor(k)[:] = v
            sim.simulate()
            results.append({k: np.array(sim.tensor(k))
                            for k in ("oboxes", "omask", "ometa", "oidx")})
        kr = None
    else:
        from concourse.bass_utils import run_bass_kernel_spmd
        kr = run_bass_kernel_spmd(nc, in_maps, list(range(8)), trace=_trace)
        results = kr.results

    merged = _merge(results)
    if merged is None:
        merged = _numpy_reference_fallback(all_rois, all_box_deltas,
                                           all_cls_scores, im_info)
    if _trace:
        return merged, kr
    return merged


# ---------------- exact numpy fallback (certification guard; never hit on
# ---------------- well-behaved inputs, kept for unconditional correctness)

def _numpy_reference_fallback(all_rois, all_box_deltas, all_cls_scores, im_info):
    deltas = all_box_deltas.reshape(B, N, C, 4)
    scores = all_cls_scores.reshape(B, N, C)
    sc = np.moveaxis(scores[:, :, 1:], 1, 2)
    valid = sc > SCORE_THR
    order = np.argsort(-np.where(valid, sc, -np.inf), axis=-1, kind="stable")
    out_boxes = np.zeros((B, MAX_DET, 4), np.float32)
    out_scores = np.zeros((B, MAX_DET), np.float32)
    out_classes = np.zeros((B, MAX_DET), np.int32)
    for b in range(B):
        Wm = np.float32(im_info[b, 1]) - 1.0
        Hm = np.float32(im_info[b, 0]) - 1.0
        cand_sc = np.full((Cm, N), -np.inf, np.float32)
        cand_bx = np.zeros((Cm, N, 4), np.float32)
        for cm in range(Cm):
            o = order[b, cm]
            r = all_rois[b, o]
            d = deltas[b, o, cm + 1]
            w = r[:, 2] - r[:, 0] + 1.0
            h = r[:, 3] - r[:, 1] + 1.0
            cx = r[:, 0] + 0.5 * w
            cy = r[:, 1] + 0.5 * h
            pcx = d[:, 0] / 10.0 * w + cx
            pcy = d[:, 1] / 10.0 * h + cy
            pw = np.exp(np.minimum(d[:, 2] / 5.0, np.float32(MAX_LOG_WH))) * w
            ph = np.exp(np.minimum(d[:, 3] / 5.0, np.float32(MAX_LOG_WH))) * h
            x1 = np.clip(pcx - 0.5 * pw, 0, Wm)
            y1 = np.clip(pcy - 0.5 * ph, 0, Hm)
            x2 = np.clip(pcx + 0.5 * pw - 1.0, 0, Wm)
            y2 = np.clip(pcy + 0.5 * ph - 1.0, 0, Hm)
            area = (x2 - x1 + 1.0) * (y2 - y1 + 1.0)
            s = sc[b, cm, o]
            keep = s > SCORE_THR
            for i in range(N):
                if not keep[i]:
                    continue
                xx1 = np.maximum(x1, x1[i]); yy1 = np.maximum(y1, y1[i])
                xx2 = np.minimum(x2, x2[i]); yy2 = np.minimum(y2, y2[i])
                inter = (np.maximum(xx2 - xx1 + 1.0, 0.0)
                         * np.maximum(yy2 - yy1 + 1.0, 0.0))
                iou = inter / (area + area[i] - inter)
                supp = (iou > NMS_THR) & (np.arange(N) > i) & keep
                keep &= ~supp
            cand_sc[cm, keep] = s[keep]
            cand_bx[cm] = np.stack([x1, y1, x2, y2], -1)
        flat = cand_sc.reshape(-1)
        o = np.argsort(-flat, kind="stable")[:MAX_DET]
        okm = np.isfinite(flat[o])
        out_scores[b] = np.where(okm, flat[o], 0.0)
        out_boxes[b] = np.where(okm[:, None], cand_bx.reshape(-1, 4)[o], 0.0)
        out_classes[b] = np.where(okm, o // N + 1, 0).astype(np.int32)
    out_batch = np.repeat(np.arange(B, dtype=np.int32), MAX_DET)
    return (out_boxes.reshape(-1, 4), out_classes.reshape(-1),
            out_scores.reshape(-1), out_batch)


# revision 40
# speedup vs baseline: 1.1015x; 1.1015x over previous
"""Trainium2 Bass kernel for nn_DetectionOutput (decode + per-class NMS + top-k).

Sharding: 8 cores = 4 images x 2 class-halves. Core c handles image b=c//2,
classes cm in [40h, 40h+40) where h=c%2 (cm = class-1, i.e. background dropped).

Algorithm (exact, certified): with uniform scores the per-image top-100 cutoff
is ~0.999 while the 25th-best score of any class is <=0.993, so only the top
KN=24 boxes per class can reach the output. Greedy-NMS keep of a sorted prefix
depends only on that prefix, so each core:
  1. 4 max8 rounds per class -> top-24 scores+indices, rank-24 cert value
     (DVE max8/find_index8/match_replace; ties resolve index-ascending,
     matching jnp stable argsort)
  2. per-round GPSIMD indirect_copy gathers of roi+delta planes for the 8
     freshly selected boxes, hidden under the next round's DVE work
  3. decode + clip boxes                            (DVE + one ACT exp)
  4. 24x24 IoU>0.7 strict-lower suppression matrix  (DVE broadcast ops)
  5. NMS keep = fixpoint of k <- valid & ~(M k), 3 iterations (covers
     suppression-chain depth 2; measured depth on this data is 1), with the
     k3==k2 residual shipped to the host
  6. emits kept-masked scores, boxes, cert + convergence meta
Host merges the two half-image candidate sets per image with the reference
tie-break (score desc, candidate index asc), certifies the prefix bound
(tau_100 > max cert, margin ~0.007 on uniform scores) and the fixpoint
residual; an exact numpy fallback runs if either check ever fails, so the
kernel is exact for any input.
"""
import sys

sys.path.insert(0, "/opt/trn_rl_repo")

import numpy as np
import concourse.bass as bass
import concourse.mybir as mybir
from concourse.tile import TileContext

F32 = mybir.dt.float32
U32 = mybir.dt.uint32
U16 = mybir.dt.uint16
Alu = mybir.AluOpType
Act = mybir.ActivationFunctionType

B, N, C = 4, 2000, 81
Cm = C - 1
P = 40            # class-problems per core
NPAD = 2048
K = 32            # scores extracted per class (4 max8 rounds)
KN = 24           # NMS prefix = gathered slots; cert score = rank KN
NROUND = K // 8       # top-K extracted; prefix = first KN, cert = vals[:, KN]
NT = 5            # gather tile-groups of 8 problems
GW = NT * KN      # gather scratch cols
MAX_DET = 100
SCORE_THR = 0.01
NMS_THR = 0.7
MAX_LOG_WH = float(np.log(1000.0 / 16.0))
NEG = -1.0e30

_CACHED_NC = None


def _patch_tile_tail_drain():
    """This walrus build rejects CTRL instructions carrying >2 sync waits
    (NCC_INLA001 'Too many sync wait commands' on the Tile tail drain).
    Emit sync-engine NOPs before the drain and spread the waits out, one
    per instruction."""
    import concourse.tile as tile_mod
    from concourse.vector_clock import ScopedClock

    if getattr(tile_mod.TileContext, "_tail_drain_patched", False):
        return

    def _drain_and_barrier(self, tick_clock, wait_clock):
        nc = self.nc
        nops = [nc.sync.nop(nofuse=True) for _ in range(30)]
        drain_inst = nc.sync.drain()
        wait_clock.add_sem_waits(
            drain_inst.ins, ScopedClock({None: tick_clock.global_clock})
        )
        waits = list(drain_inst.ins.sync_info.on_wait or [])
        if len(waits) > 1:
            assert len(waits) <= len(nops) + 1
            drain_inst.ins.sync_info.on_wait = [waits[0]]
            for w, nop in zip(waits[1:], nops):
                nop.ins.sync_info = mybir.SyncInfo(on_wait=[w], on_update=[])
        nc.all_engine_barrier()
        assert self.sems is not None
        popped = nc._tile_sem_poison_stack.pop()
        assert popped is self._sem_poison
        nc.clear_and_free_semaphores(list(self.sems.allocated().values()))
        nc.all_engine_barrier()

    tile_mod.TileContext._drain_and_barrier = _drain_and_barrier
    tile_mod.TileContext._tail_drain_patched = True


def _split_sync_waits(nc, max_waits=1):
    """Walrus codegen in this container rejects instructions carrying more
    than 1-2 sync waits (class-dependent). Cap every instruction at
    `max_waits` by hoisting the excess onto same-engine NoOps inserted
    immediately before it (engine blocks at the same program point, so
    semantics and Tile's schedule-order guarantees are preserved)."""
    import bass_rust
    ctr = 0
    for f in nc.m.functions:
        for bb in f.blocks:
            ins_list = bb.instructions
            new = []
            for inst in ins_list:
                si = inst.sync_info
                waits = list(si.on_wait) if si and si.on_wait else []
                if len(waits) > max_waits:
                    for w in waits[max_waits:]:
                        ctr += 1
                        nop = bass_rust.InstNoOp(name=f"WSPLIT-{ctr}")
                        nop.engine = inst.engine
                        nop.sync_info = mybir.SyncInfo(on_wait=[w], on_update=[])
                        new.append(nop)
                    inst.sync_info = mybir.SyncInfo(
                        on_wait=waits[:max_waits],
                        on_update=list(si.on_update) if si.on_update else [])
                new.append(inst)
            ins_list[:] = new
    return ctr


def build_program(split_waits=True):
    _patch_tile_tail_drain()
    nc = bass.Bass()
    sc_in = nc.dram_tensor("sc_in", [P, NPAD], F32, kind="ExternalInput")
    planes_in = nc.dram_tensor("planes_in", [NT, 128, N], F32, kind="ExternalInput")
    clip_in = nc.dram_tensor("clip_in", [P, 2], F32, kind="ExternalInput")  # [Wm, Hm]
    oboxes = nc.dram_tensor("oboxes", [4, P, KN], F32, kind="ExternalOutput")
    omask = nc.dram_tensor("omask", [P, KN], F32, kind="ExternalOutput")
    ometa = nc.dram_tensor("ometa", [P, 40], F32, kind="ExternalOutput")
    tri_in = nc.dram_tensor("tri_in", [KN * KN], F32, kind="ExternalInput")
    oidx = nc.dram_tensor("oidx", [P, KN], U32, kind="ExternalOutput")
    scr_gat = nc.dram_tensor("scr_gat", [128 * GW], F32)

    with TileContext(nc) as tc:
        with (
            tc.tile_pool(name="pool", bufs=1) as pool,
            tc.tile_pool(name="plpool", bufs=1) as plpool,
        ):
            sc = pool.tile([P, NPAD], F32)
            nc.sync.dma_start(sc[:], sc_in[:])
            clip = pool.tile([P, 2], F32)
            nc.sync.dma_start(clip[:], clip_in[:])

            # plane tiles up-front; their DMAs overlap the topk rounds
            pls = []
            for t in range(NT):
                pl = plpool.tile([128, N], F32, tag=f"pl{t}", name=f"pl{t}")
                nc.sync.dma_start(pl[:], planes_in[t])
                pls.append(pl)

            # ---- stage 1+2: topk rounds with per-round gather ----
            # idxr row p holds round r's 8 indices at cols 16r..16r+8 (u16);
            # iwt_rt[16g+j, 0] = idxr[8t+g, 16r+j] so indirect_copy's "(s p)"
            # unwrap yields slots 8r..8r+7 in order. Gathers for round r run
            # on GPSIMD under round r+1's DVE work.
            vals = pool.tile([P, NROUND * 8], F32)
            idx = pool.tile([P, NROUND * 8], U32)
            idxr = pool.tile([P, 16 * NROUND], U16)
            nc.vector.memset(idxr[:], 0)
            gat = pool.tile([128, NT * KN], F32)
            irap = idxr[:]
            pk = []
            for k in range(8):
                pkt = pool.tile([P, KN], F32, tag=f"pk{k}", name=f"pk{k}")
                pk.append(pkt)
            for r in range(NROUND):
                s8 = slice(r * 8, (r + 1) * 8)
                nc.vector.max(vals[:, s8], sc[:])
                if r * 8 >= KN:
                    continue    # cert-only round: no indices, no gather
                nc.vector.max_index(idx[:, s8], vals[:, s8], sc[:])
                if r + 1 < NROUND:
                    nc.vector.match_replace(sc[:], vals[:, s8], sc[:], NEG)
                nc.vector.tensor_copy(idxr[:, 16 * r:16 * r + 8], idx[:, s8])
                for t in range(NT):
                    iwt = plpool.tile([128, 1], U16, tag=f"iw{t}_{r}",
                                      name=f"iw{t}_{r}")
                    nc.sync.dma_start(
                        iwt[:], bass.AP(irap.tensor,
                                        irap.offset + 8 * t * 16 * NROUND + 16 * r,
                                        [[16 * NROUND, 8], [1, 16], [1, 1]]))
                    nc.gpsimd.indirect_copy(
                        gat[:, t * KN + 8 * r:t * KN + 8 * r + 8].rearrange(
                            "p (i one) -> p i one", one=1),
                        pls[t][:], iwt[:], True)
                # bounce this round's gathered columns to DRAM for un-interleave
                gap = gat[:]
                nc.sync.dma_start(
                    bass.AP(scr_gat, 8 * r, [[GW, 128], [KN, NT], [1, 8]]),
                    bass.AP(gap.tensor, gap.offset + 8 * r,
                            [[GW, 128], [KN, NT], [1, 8]]))
            nc.sync.dma_start(ometa[:, 0:NROUND * 8], vals[:])
            nc.sync.dma_start(oidx[:], idx[:, :KN])
            for k in range(8):
                nc.sync.dma_start(
                    pk[k][:], bass.AP(scr_gat, k * GW, [[KN, NT], [16 * GW, 8], [1, KN]]))
            x1r, y1r, x2r, y2r, dx, dy, dw, dh = pk

            _tagn = [0]

            def tile():
                _tagn[0] += 1
                return pool.tile([P, KN], F32, tag=f"dec{_tagn[0]}", name=f"dec{_tagn[0]}")

            V = nc.vector
            # ---- stage 3: decode (mirrors reference fp op order) ----
            w = pool.tile([P, KN], F32, tag="w")
            h = pool.tile([P, KN], F32, tag="h")
            cx, cy, pcx, pcy, pw, ph, area = (tile() for _ in range(7))
            ew = pool.tile([P, 2 * KN], F32, tag="ew")
            aw = pool.tile([P, KN], F32, tag="aw")
            x1o, y1o, x2o, y2o = x1r, y1r, x2r, y2r

            def decode_wave(sl):
                s2 = slice(KN + sl.start, KN + sl.stop)
                V.tensor_tensor(w[:, sl], x2r[:, sl], x1r[:, sl], Alu.subtract)
                V.tensor_scalar_add(w[:, sl], w[:, sl], 1.0)
                V.tensor_tensor(h[:, sl], y2r[:, sl], y1r[:, sl], Alu.subtract)
                V.tensor_scalar_add(h[:, sl], h[:, sl], 1.0)
                V.tensor_scalar_mul(cx[:, sl], w[:, sl], 0.5)
                V.tensor_tensor(cx[:, sl], x1r[:, sl], cx[:, sl], Alu.add)
                V.tensor_scalar_mul(cy[:, sl], h[:, sl], 0.5)
                V.tensor_tensor(cy[:, sl], y1r[:, sl], cy[:, sl], Alu.add)
                V.tensor_scalar_mul(pcx[:, sl], dx[:, sl], 0.1)
                V.tensor_tensor(pcx[:, sl], pcx[:, sl], w[:, sl], Alu.mult)
                V.tensor_tensor(pcx[:, sl], pcx[:, sl], cx[:, sl], Alu.add)
                V.tensor_scalar_mul(pcy[:, sl], dy[:, sl], 0.1)
                V.tensor_tensor(pcy[:, sl], pcy[:, sl], h[:, sl], Alu.mult)
                V.tensor_tensor(pcy[:, sl], pcy[:, sl], cy[:, sl], Alu.add)
                V.tensor_scalar(ew[:, sl], dw[:, sl], 0.2, MAX_LOG_WH,
                                Alu.mult, Alu.min)
                V.tensor_scalar(ew[:, s2], dh[:, sl], 0.2, MAX_LOG_WH,
                                Alu.mult, Alu.min)
                nc.scalar.activation(ew[:, sl], ew[:, sl], Act.Exp)
                nc.scalar.activation(ew[:, s2], ew[:, s2], Act.Exp)
                V.tensor_tensor(pw[:, sl], ew[:, sl], w[:, sl], Alu.mult)
                V.tensor_tensor(ph[:, sl], ew[:, s2], h[:, sl], Alu.mult)
                V.tensor_scalar_mul(pw[:, sl], pw[:, sl], 0.5)
                V.tensor_tensor(x1o[:, sl], pcx[:, sl], pw[:, sl], Alu.subtract)
                V.tensor_tensor(x2o[:, sl], pcx[:, sl], pw[:, sl], Alu.add)
                V.tensor_scalar_add(x2o[:, sl], x2o[:, sl], -1.0)
                V.tensor_scalar_mul(ph[:, sl], ph[:, sl], 0.5)
                V.tensor_tensor(y1o[:, sl], pcy[:, sl], ph[:, sl], Alu.subtract)
                V.tensor_tensor(y2o[:, sl], pcy[:, sl], ph[:, sl], Alu.add)
                V.tensor_scalar_add(y2o[:, sl], y2o[:, sl], -1.0)
                for tl, cc in ((x1o, 0), (x2o, 0), (y1o, 1), (y2o, 1)):
                    V.tensor_scalar(tl[:, sl], tl[:, sl], 0.0,
                                    clip[:, cc:cc + 1], Alu.max, Alu.min)
                V.tensor_tensor(aw[:, sl], x2o[:, sl], x1o[:, sl], Alu.subtract)
                V.tensor_scalar_add(aw[:, sl], aw[:, sl], 1.0)
                V.tensor_tensor(area[:, sl], y2o[:, sl], y1o[:, sl], Alu.subtract)
                V.tensor_scalar_add(area[:, sl], area[:, sl], 1.0)
                V.tensor_tensor(area[:, sl], area[:, sl], aw[:, sl], Alu.mult)

            decode_wave(slice(0, KN))

            # ---- stage 4: suppression matrix M[p, i, j] = IoU(i,j) > thr ----
            def iview(t):
                return t[:, :, None].broadcast_to([P, KN, KN])

            def jview(t):
                return t[:, None, :].broadcast_to([P, KN, KN])

            def big():
                _tagn[0] += 1
                tl = pool.tile([P, KN * KN], F32, tag=f"big{_tagn[0]}", name=f"big{_tagn[0]}")
                return tl, tl[:].rearrange("p (i j) -> p i j", i=KN)

            M, Mv = big()
            w_t, w_v = big()
            xx1, xx1v = big()
            V.tensor_tensor(xx1v, iview(x1o), jview(x1o), Alu.max)
            V.tensor_tensor(Mv, iview(x2o), jview(x2o), Alu.min)
            V.tensor_tensor(w_v, Mv, xx1v, Alu.subtract)
            V.tensor_scalar(w_t[:], w_t[:], 1.0, 0.0, Alu.add, Alu.max)
            V.tensor_tensor(xx1v, iview(y1o), jview(y1o), Alu.max)
            V.tensor_tensor(Mv, iview(y2o), jview(y2o), Alu.min)
            V.tensor_tensor(xx1v, Mv, xx1v, Alu.subtract)
            V.tensor_scalar(xx1[:], xx1[:], 1.0, 0.0, Alu.add, Alu.max)
            inter = w_t
            V.tensor_tensor(inter[:], w_t[:], xx1[:], Alu.mult)   # inter
            V.tensor_tensor(Mv, jview(area), iview(area), Alu.add)
            V.tensor_tensor(M[:], M[:], inter[:], Alu.subtract)    # union
            V.tensor_scalar_mul(M[:], M[:], NMS_THR)
            V.tensor_tensor(M[:], inter[:], M[:], Alu.is_gt)       # M flags

            # ---- stage 5: NMS via fixpoint iteration ----
            # M[p, a, b] = (IoU(a,b) > thr) & (b < a): k_{t+1}[a] =
            # valid[a] & ~any_b(M[a,b] & k_t[b]). The greedy keep set is the
            # unique fixpoint; 2 iterations cover suppression-chain depth 1
            # (measured depth on this data: 1) and the k4==k3 convergence
            # residual ships to the host, which falls back to an exact
            # reference recompute if it is ever nonzero.
            TRI = pool.tile([P, KN * KN], F32, tag="TRI")
            nc.sync.dma_start(TRI[:], bass.AP(tri_in, 0, [[0, P], [1, KN * KN]]))
            V.tensor_tensor(M[:], M[:], TRI[:], Alu.mult)
            valid = pool.tile([P, KN], F32, tag="valid")
            V.tensor_single_scalar(valid[:], vals[:, :KN], SCORE_THR, Alu.is_gt)
            ka = pool.tile([P, KN], F32, tag="ka")
            kb = pool.tile([P, KN], F32, tag="kb")
            supp = pool.tile([P, KN], F32, tag="supp")
            TMP, TMPv = big()
            k_prev, k_cur = None, valid
            for it in range(2):
                k_next = ka if it % 2 == 0 else kb
                V.tensor_tensor(
                    TMPv, Mv,
                    k_cur[:, None, :].broadcast_to([P, KN, KN]), Alu.mult)
                V.tensor_reduce(supp[:], TMPv, mybir.AxisListType.X, Alu.max)
                V.scalar_tensor_tensor(k_next[:], supp[:], 0.0, valid[:],
                                       Alu.is_equal, Alu.mult)
                k_prev, k_cur = k_cur, k_next
            dtile = pool.tile([P, KN], F32, tag="dtile")
            V.tensor_tensor(dtile[:], k_cur[:], k_prev[:], Alu.not_equal)
            dsum = pool.tile([P, 1], F32, tag="dsum")
            V.tensor_reduce(dsum[:], dtile[:], mybir.AxisListType.X, Alu.add)
            nc.sync.dma_start(ometa[:, 32:33], dsum[:])

            # ---- stage 6: masked scores + outputs ----
            good = k_cur
            pen = pool.tile([P, KN], F32, tag="pen")
            V.tensor_scalar(pen[:], good[:], -NEG, NEG, Alu.mult, Alu.add)  # 0 kept, NEG else
            V.tensor_tensor(good[:], good[:], vals[:, :KN], Alu.mult)
            V.tensor_tensor(good[:], good[:], pen[:], Alu.add)
            nc.sync.dma_start(omask[:], good[:])
            for kk, tl in enumerate((x1o, y1o, x2o, y2o)):
                nc.sync.dma_start(oboxes[kk], tl[:])
    if split_waits:
        _split_sync_waits(nc)
    return nc


# ---------------------------------------------------------------- host side

def _prep_core(all_rois, all_box_deltas, all_cls_scores, im_info, core):
    b, h = core // 2, core % 2
    sc = np.full((P, NPAD), NEG, np.float32)
    sc[:, :N] = all_cls_scores.reshape(B, N, C)[b, :, 1 + 40 * h:41 + 40 * h].T
    planes = np.zeros((NT, 8, 16, N), np.float32)
    planes[:, :, 0:4, :] = all_rois[b].T[None, None]
    dsl = all_box_deltas.reshape(B, N, C * 4)[b][:, 4 + 160 * h:164 + 160 * h]
    planes[:, :, 4:8, :] = dsl.T.reshape(NT, 8, 4, N)
    clip = np.empty((P, 2), np.float32)
    clip[:, 0] = np.float32(im_info[b, 1]) - np.float32(1.0)
    clip[:, 1] = np.float32(im_info[b, 0]) - np.float32(1.0)
    a = np.arange(KN)
    tri = (a[None, :] < a[:, None]).astype(np.float32).reshape(-1)
    return {"sc_in": np.ascontiguousarray(sc),
            "planes_in": np.ascontiguousarray(planes.reshape(NT, 128, N)),
            "clip_in": clip, "tri_in": tri}


def _merge(results):
    out_boxes = np.zeros((B, MAX_DET, 4), np.float32)
    out_scores = np.zeros((B, MAX_DET), np.float32)
    out_classes = np.zeros((B, MAX_DET), np.int32)
    ok_all = True
    for b in range(B):
        r0, r1 = results[2 * b], results[2 * b + 1]
        masked = np.concatenate([r0["omask"], r1["omask"]], axis=0)      # [80, K]
        boxes = np.concatenate([r0["oboxes"], r1["oboxes"]], axis=1)     # [4, 80, K]
        cert = max(float(r0["ometa"][:, KN].max()), float(r1["ometa"][:, KN].max()))
        if float(r0["ometa"][:, 32].sum()) != 0.0 or float(r1["ometa"][:, 32].sum()) != 0.0:
            ok_all = False
            break
        flat = masked.reshape(-1)
        order = np.argsort(-flat, kind="stable")[:MAX_DET]
        ssel = flat[order]
        if not (ssel[-1] > cert and ssel[-1] > -1.0e29):
            ok_all = False
            break
        cm_sel = order // KN
        out_scores[b] = ssel
        out_classes[b] = (cm_sel + 1).astype(np.int32)
        bt = boxes.reshape(4, -1)
        out_boxes[b] = bt[:, order].T
    if not ok_all:
        return None
    out_batch = np.repeat(np.arange(B, dtype=np.int32), MAX_DET)
    return (out_boxes.reshape(-1, 4), out_classes.reshape(-1),
            out_scores.reshape(-1), out_batch)


def kernel(all_rois, all_box_deltas, all_cls_scores, im_info, _sim=False, _trace=False):
    global _CACHED_NC
    all_rois = np.asarray(all_rois, np.float32)
    all_box_deltas = np.asarray(all_box_deltas, np.float32)
    all_cls_scores = np.asarray(all_cls_scores, np.float32)
    im_info = np.asarray(im_info, np.float32)

    if _sim:
        nc = build_program(split_waits=False)
    else:
        if _CACHED_NC is None:
            _CACHED_NC = build_program()
        nc = _CACHED_NC
    in_maps = [_prep_core(all_rois, all_box_deltas, all_cls_scores, im_info, c)
               for c in range(8)]

    if _sim:
        from concourse import bass_interp
        results = []
        for c in range(8):
            sim = bass_interp.CoreSim(nc)
            for k, v in in_maps[c].items():
                sim.tensor(k)[:] = v
            sim.simulate()
            results.append({k: np.array(sim.tensor(k))
                            for k in ("oboxes", "omask", "ometa", "oidx")})
        kr = None
    else:
        from concourse.bass_utils import run_bass_kernel_spmd
        kr = run_bass_kernel_spmd(nc, in_maps, list(range(8)), trace=_trace)
        results = kr.results

    merged = _merge(results)
    if merged is None:
        merged = _numpy_reference_fallback(all_rois, all_box_deltas,
                                           all_cls_scores, im_info)
    if _trace:
        return merged, kr
    return merged


# ---------------- exact numpy fallback (certification guard; never hit on
# ---------------- well-behaved inputs, kept for unconditional correctness)

def _numpy_reference_fallback(all_rois, all_box_deltas, all_cls_scores, im_info):
    deltas = all_box_deltas.reshape(B, N, C, 4)
    scores = all_cls_scores.reshape(B, N, C)
    sc = np.moveaxis(scores[:, :, 1:], 1, 2)
    valid = sc > SCORE_THR
    order = np.argsort(-np.where(valid, sc, -np.inf), axis=-1, kind="stable")
    out_boxes = np.zeros((B, MAX_DET, 4), np.float32)
    out_scores = np.zeros((B, MAX_DET), np.float32)
    out_classes = np.zeros((B, MAX_DET), np.int32)
    for b in range(B):
        Wm = np.float32(im_info[b, 1]) - 1.0
        Hm = np.float32(im_info[b, 0]) - 1.0
        cand_sc = np.full((Cm, N), -np.inf, np.float32)
        cand_bx = np.zeros((Cm, N, 4), np.float32)
        for cm in range(Cm):
            o = order[b, cm]
            r = all_rois[b, o]
            d = deltas[b, o, cm + 1]
            w = r[:, 2] - r[:, 0] + 1.0
            h = r[:, 3] - r[:, 1] + 1.0
            cx = r[:, 0] + 0.5 * w
            cy = r[:, 1] + 0.5 * h
            pcx = d[:, 0] / 10.0 * w + cx
            pcy = d[:, 1] / 10.0 * h + cy
            pw = np.exp(np.minimum(d[:, 2] / 5.0, np.float32(MAX_LOG_WH))) * w
            ph = np.exp(np.minimum(d[:, 3] / 5.0, np.float32(MAX_LOG_WH))) * h
            x1 = np.clip(pcx - 0.5 * pw, 0, Wm)
            y1 = np.clip(pcy - 0.5 * ph, 0, Hm)
            x2 = np.clip(pcx + 0.5 * pw - 1.0, 0, Wm)
            y2 = np.clip(pcy + 0.5 * ph - 1.0, 0, Hm)
            area = (x2 - x1 + 1.0) * (y2 - y1 + 1.0)
            s = sc[b, cm, o]
            keep = s > SCORE_THR
            for i in range(N):
                if not keep[i]:
                    continue
                xx1 = np.maximum(x1, x1[i]); yy1 = np.maximum(y1, y1[i])
                xx2 = np.minimum(x2, x2[i]); yy2 = np.minimum(y2, y2[i])
                inter = (np.maximum(xx2 - xx1 + 1.0, 0.0)
                         * np.maximum(yy2 - yy1 + 1.0, 0.0))
                iou = inter / (area + area[i] - inter)
                supp = (iou > NMS_THR) & (np.arange(N) > i) & keep
                keep &= ~supp
            cand_sc[cm, keep] = s[keep]
            cand_bx[cm] = np.stack([x1, y1, x2, y2], -1)
        flat = cand_sc.reshape(-1)
        o = np.argsort(-flat, kind="stable")[:MAX_DET]
        okm = np.isfinite(flat[o])
        out_scores[b] = np.where(okm, flat[o], 0.0)
        out_boxes[b] = np.where(okm[:, None], cand_bx.reshape(-1, 4)[o], 0.0)
        out_classes[b] = np.where(okm, o // N + 1, 0).astype(np.int32)
    out_batch = np.repeat(np.arange(B, dtype=np.int32), MAX_DET)
    return (out_boxes.reshape(-1, 4), out_classes.reshape(-1),
            out_scores.reshape(-1), out_batch)


# revision 41
# speedup vs baseline: 1.1319x; 1.0276x over previous
"""Trainium2 Bass kernel for nn_DetectionOutput (decode + per-class NMS + top-k).

Sharding: 8 cores = 4 images x 2 class-halves. Core c handles image b=c//2,
classes cm in [40h, 40h+40) where h=c%2 (cm = class-1, i.e. background dropped).

Algorithm (exact, certified): with uniform scores the per-image top-100 cutoff
is ~0.999 while the 25th-best score of any class is <=0.993, so only the top
KN=24 boxes per class can reach the output. Greedy-NMS keep of a sorted prefix
depends only on that prefix, so each core:
  1. 4 max8 rounds per class -> top-24 scores+indices, rank-24 cert value
     (DVE max8/find_index8/match_replace; ties resolve index-ascending,
     matching jnp stable argsort)
  2. per-round GPSIMD indirect_copy gathers of roi+delta planes for the 8
     freshly selected boxes, hidden under the next round's DVE work
  3. decode + clip boxes                            (DVE + one ACT exp)
  4. 24x24 IoU>0.7 strict-lower suppression matrix  (DVE broadcast ops)
  5. NMS keep = fixpoint of k <- valid & ~(M k), 3 iterations (covers
     suppression-chain depth 2; measured depth on this data is 1), with the
     k3==k2 residual shipped to the host
  6. emits kept-masked scores, boxes, cert + convergence meta
Host merges the two half-image candidate sets per image with the reference
tie-break (score desc, candidate index asc), certifies the prefix bound
(tau_100 > max cert, margin ~0.007 on uniform scores) and the fixpoint
residual; an exact numpy fallback runs if either check ever fails, so the
kernel is exact for any input.
"""
import sys

sys.path.insert(0, "/opt/trn_rl_repo")

import numpy as np
import concourse.bass as bass
import concourse.mybir as mybir
from concourse.tile import TileContext

F32 = mybir.dt.float32
U32 = mybir.dt.uint32
U16 = mybir.dt.uint16
Alu = mybir.AluOpType
Act = mybir.ActivationFunctionType

B, N, C = 4, 2000, 81
Cm = C - 1
P = 40            # class-problems per core
NPAD = 2048
K = 32            # scores extracted per class (4 max8 rounds)
KN = 24           # NMS prefix = gathered slots; cert score = rank KN
NROUND = K // 8       # top-K extracted; prefix = first KN, cert = vals[:, KN]
NT = 5            # gather tile-groups of 8 problems
GW = NT * KN      # gather scratch cols
MAX_DET = 100
SCORE_THR = 0.01
NMS_THR = 0.7
MAX_LOG_WH = float(np.log(1000.0 / 16.0))
NEG = -1.0e30

_CACHED_NC = None


def _patch_tile_tail_drain():
    """This walrus build rejects CTRL instructions carrying >2 sync waits
    (NCC_INLA001 'Too many sync wait commands' on the Tile tail drain).
    Emit sync-engine NOPs before the drain and spread the waits out, one
    per instruction."""
    import concourse.tile as tile_mod
    from concourse.vector_clock import ScopedClock

    if getattr(tile_mod.TileContext, "_tail_drain_patched", False):
        return

    def _drain_and_barrier(self, tick_clock, wait_clock):
        nc = self.nc
        nops = [nc.sync.nop(nofuse=True) for _ in range(30)]
        drain_inst = nc.sync.drain()
        wait_clock.add_sem_waits(
            drain_inst.ins, ScopedClock({None: tick_clock.global_clock})
        )
        waits = list(drain_inst.ins.sync_info.on_wait or [])
        if len(waits) > 1:
            assert len(waits) <= len(nops) + 1
            drain_inst.ins.sync_info.on_wait = [waits[0]]
            for w, nop in zip(waits[1:], nops):
                nop.ins.sync_info = mybir.SyncInfo(on_wait=[w], on_update=[])
        nc.all_engine_barrier()
        assert self.sems is not None
        popped = nc._tile_sem_poison_stack.pop()
        assert popped is self._sem_poison
        nc.clear_and_free_semaphores(list(self.sems.allocated().values()))
        nc.all_engine_barrier()

    tile_mod.TileContext._drain_and_barrier = _drain_and_barrier
    tile_mod.TileContext._tail_drain_patched = True


def _split_sync_waits(nc, max_waits=1):
    """Walrus codegen in this container rejects instructions carrying more
    than 1-2 sync waits (class-dependent). Cap every instruction at
    `max_waits` by hoisting the excess onto same-engine NoOps inserted
    immediately before it (engine blocks at the same program point, so
    semantics and Tile's schedule-order guarantees are preserved)."""
    import bass_rust
    ctr = 0
    for f in nc.m.functions:
        for bb in f.blocks:
            ins_list = bb.instructions
            new = []
            for inst in ins_list:
                si = inst.sync_info
                waits = list(si.on_wait) if si and si.on_wait else []
                if len(waits) > max_waits:
                    for w in waits[max_waits:]:
                        ctr += 1
                        nop = bass_rust.InstNoOp(name=f"WSPLIT-{ctr}")
                        nop.engine = inst.engine
                        nop.sync_info = mybir.SyncInfo(on_wait=[w], on_update=[])
                        new.append(nop)
                    inst.sync_info = mybir.SyncInfo(
                        on_wait=waits[:max_waits],
                        on_update=list(si.on_update) if si.on_update else [])
                new.append(inst)
            ins_list[:] = new
    return ctr


def build_program(split_waits=True):
    _patch_tile_tail_drain()
    nc = bass.Bass()
    sc_in = nc.dram_tensor("sc_in", [P, NPAD], F32, kind="ExternalInput")
    planes_in = nc.dram_tensor("planes_in", [NT, 128, N], F32, kind="ExternalInput")
    clip_in = nc.dram_tensor("clip_in", [P, 2], F32, kind="ExternalInput")  # [Wm, Hm]
    oboxes = nc.dram_tensor("oboxes", [4, P, KN], F32, kind="ExternalOutput")
    omask = nc.dram_tensor("omask", [P, KN], F32, kind="ExternalOutput")
    ometa = nc.dram_tensor("ometa", [P, 40], F32, kind="ExternalOutput")
    tri_in = nc.dram_tensor("tri_in", [KN * KN], F32, kind="ExternalInput")
    oidx = nc.dram_tensor("oidx", [P, KN], U32, kind="ExternalOutput")
    scr_gat = nc.dram_tensor("scr_gat", [128 * GW], F32)

    with TileContext(nc) as tc:
        with (
            tc.tile_pool(name="pool", bufs=1) as pool,
            tc.tile_pool(name="plpool", bufs=1) as plpool,
        ):
            sc = pool.tile([P, NPAD], F32)
            nc.sync.dma_start(sc[:], sc_in[:])
            clip = pool.tile([P, 2], F32)
            nc.sync.dma_start(clip[:], clip_in[:])

            # plane tiles up-front; their DMAs overlap the topk rounds
            pls = []
            for t in range(NT):
                pl = plpool.tile([128, N], F32, tag=f"pl{t}", name=f"pl{t}")
                nc.sync.dma_start(pl[:], planes_in[t])
                pls.append(pl)

            # ---- stage 1+2: topk rounds with per-round gather ----
            # idxr row p holds round r's 8 indices at cols 16r..16r+8 (u16);
            # iwt_rt[16g+j, 0] = idxr[8t+g, 16r+j] so indirect_copy's "(s p)"
            # unwrap yields slots 8r..8r+7 in order. Gathers for round r run
            # on GPSIMD under round r+1's DVE work.
            vals = pool.tile([P, NROUND * 8], F32)
            idx = pool.tile([P, NROUND * 8], U32)
            idxr = pool.tile([P, 16 * NROUND], U16)
            nc.vector.memset(idxr[:], 0)
            gat = pool.tile([128, NT * KN], F32)
            irap = idxr[:]
            pk = []
            for k in range(8):
                pkt = pool.tile([P, KN], F32, tag=f"pk{k}", name=f"pk{k}")
                pk.append(pkt)
            for r in range(NROUND):
                s8 = slice(r * 8, (r + 1) * 8)
                nc.vector.max(vals[:, s8], sc[:])
                if r * 8 >= KN:
                    continue    # cert-only round: no indices, no gather
                nc.vector.max_index(idx[:, s8], vals[:, s8], sc[:])
                if r + 1 < NROUND:
                    nc.vector.match_replace(sc[:], vals[:, s8], sc[:], NEG)
                nc.vector.tensor_copy(idxr[:, 16 * r:16 * r + 8], idx[:, s8])
                for t in range(NT):
                    iwt = plpool.tile([128, 1], U16, tag=f"iw{t}_{r}",
                                      name=f"iw{t}_{r}")
                    nc.sync.dma_start(
                        iwt[:], bass.AP(irap.tensor,
                                        irap.offset + 8 * t * 16 * NROUND + 16 * r,
                                        [[16 * NROUND, 8], [1, 16], [1, 1]]))
                    nc.gpsimd.indirect_copy(
                        gat[:, t * KN + 8 * r:t * KN + 8 * r + 8].rearrange(
                            "p (i one) -> p i one", one=1),
                        pls[t][:], iwt[:], True)
                # bounce this round's gathered columns to DRAM for un-interleave
                gap = gat[:]
                nc.sync.dma_start(
                    bass.AP(scr_gat, 8 * r, [[GW, 128], [KN, NT], [1, 8]]),
                    bass.AP(gap.tensor, gap.offset + 8 * r,
                            [[GW, 128], [KN, NT], [1, 8]]))
            nc.sync.dma_start(ometa[:, 0:NROUND * 8], vals[:])
            nc.sync.dma_start(oidx[:], idx[:, :KN])
            for k in range(8):
                nc.sync.dma_start(
                    pk[k][:], bass.AP(scr_gat, k * GW, [[KN, NT], [16 * GW, 8], [1, KN]]))
            x1r, y1r, x2r, y2r, dx, dy, dw, dh = pk

            _tagn = [0]

            def tile():
                _tagn[0] += 1
                return pool.tile([P, KN], F32, tag=f"dec{_tagn[0]}", name=f"dec{_tagn[0]}")

            V = nc.vector
            # ---- stage 3: decode (mirrors reference fp op order) ----
            w = pool.tile([P, KN], F32, tag="w")
            h = pool.tile([P, KN], F32, tag="h")
            cx, cy, pcx, pcy, pw, ph, area = (tile() for _ in range(7))
            ew = pool.tile([P, 2 * KN], F32, tag="ew")
            aw = pool.tile([P, KN], F32, tag="aw")
            x1o, y1o, x2o, y2o = x1r, y1r, x2r, y2r

            def decode_wave(sl):
                s2 = slice(KN + sl.start, KN + sl.stop)
                V.tensor_tensor(w[:, sl], x2r[:, sl], x1r[:, sl], Alu.subtract)
                V.tensor_scalar_add(w[:, sl], w[:, sl], 1.0)
                V.tensor_tensor(h[:, sl], y2r[:, sl], y1r[:, sl], Alu.subtract)
                V.tensor_scalar_add(h[:, sl], h[:, sl], 1.0)
                V.tensor_scalar_mul(cx[:, sl], w[:, sl], 0.5)
                V.tensor_tensor(cx[:, sl], x1r[:, sl], cx[:, sl], Alu.add)
                V.tensor_scalar_mul(cy[:, sl], h[:, sl], 0.5)
                V.tensor_tensor(cy[:, sl], y1r[:, sl], cy[:, sl], Alu.add)
                V.tensor_scalar_mul(pcx[:, sl], dx[:, sl], 0.1)
                V.tensor_tensor(pcx[:, sl], pcx[:, sl], w[:, sl], Alu.mult)
                V.tensor_tensor(pcx[:, sl], pcx[:, sl], cx[:, sl], Alu.add)
                V.tensor_scalar_mul(pcy[:, sl], dy[:, sl], 0.1)
                V.tensor_tensor(pcy[:, sl], pcy[:, sl], h[:, sl], Alu.mult)
                V.tensor_tensor(pcy[:, sl], pcy[:, sl], cy[:, sl], Alu.add)
                V.tensor_scalar(ew[:, sl], dw[:, sl], 0.2, MAX_LOG_WH,
                                Alu.mult, Alu.min)
                V.tensor_scalar(ew[:, s2], dh[:, sl], 0.2, MAX_LOG_WH,
                                Alu.mult, Alu.min)
                nc.scalar.activation(ew[:, sl], ew[:, sl], Act.Exp)
                nc.scalar.activation(ew[:, s2], ew[:, s2], Act.Exp)
                V.tensor_tensor(pw[:, sl], ew[:, sl], w[:, sl], Alu.mult)
                V.tensor_tensor(ph[:, sl], ew[:, s2], h[:, sl], Alu.mult)
                V.tensor_scalar_mul(pw[:, sl], pw[:, sl], 0.5)
                V.tensor_tensor(x1o[:, sl], pcx[:, sl], pw[:, sl], Alu.subtract)
                V.tensor_tensor(x2o[:, sl], pcx[:, sl], pw[:, sl], Alu.add)
                V.tensor_scalar_add(x2o[:, sl], x2o[:, sl], -1.0)
                V.tensor_scalar_mul(ph[:, sl], ph[:, sl], 0.5)
                V.tensor_tensor(y1o[:, sl], pcy[:, sl], ph[:, sl], Alu.subtract)
                V.tensor_tensor(y2o[:, sl], pcy[:, sl], ph[:, sl], Alu.add)
                V.tensor_scalar_add(y2o[:, sl], y2o[:, sl], -1.0)
                for tl, cc in ((x1o, 0), (x2o, 0), (y1o, 1), (y2o, 1)):
                    V.tensor_scalar(tl[:, sl], tl[:, sl], 0.0,
                                    clip[:, cc:cc + 1], Alu.max, Alu.min)
                V.tensor_tensor(aw[:, sl], x2o[:, sl], x1o[:, sl], Alu.subtract)
                V.tensor_scalar_add(aw[:, sl], aw[:, sl], 1.0)
                V.tensor_tensor(area[:, sl], y2o[:, sl], y1o[:, sl], Alu.subtract)
                V.tensor_scalar_add(area[:, sl], area[:, sl], 1.0)
                V.tensor_tensor(area[:, sl], area[:, sl], aw[:, sl], Alu.mult)

            decode_wave(slice(0, KN))

            # ---- stage 4: suppression matrix M[p, i, j] = IoU(i,j) > thr ----
            def iview(t):
                return t[:, :, None].broadcast_to([P, KN, KN])

            def jview(t):
                return t[:, None, :].broadcast_to([P, KN, KN])

            def big():
                _tagn[0] += 1
                tl = pool.tile([P, KN * KN], F32, tag=f"big{_tagn[0]}", name=f"big{_tagn[0]}")
                return tl, tl[:].rearrange("p (i j) -> p i j", i=KN)

            M, Mv = big()
            w_t, w_v = big()
            xx1, xx1v = big()
            V.tensor_tensor(xx1v, iview(x1o), jview(x1o), Alu.max)
            V.tensor_tensor(Mv, iview(x2o), jview(x2o), Alu.min)
            V.tensor_tensor(w_v, Mv, xx1v, Alu.subtract)
            V.tensor_scalar(w_t[:], w_t[:], 1.0, 0.0, Alu.add, Alu.max)
            V.tensor_tensor(xx1v, iview(y1o), jview(y1o), Alu.max)
            V.tensor_tensor(Mv, iview(y2o), jview(y2o), Alu.min)
            V.tensor_tensor(xx1v, Mv, xx1v, Alu.subtract)
            V.tensor_scalar(xx1[:], xx1[:], 1.0, 0.0, Alu.add, Alu.max)
            inter = w_t
            V.tensor_tensor(inter[:], w_t[:], xx1[:], Alu.mult)   # inter
            # IoU > t  <=>  inter > (t/(1+t))*(Ai+Aj); margin |IoU-0.7| on
            # this data is 2e-5 >> the ~1e-7 fp discrepancy of the rewrite
            V.tensor_scalar_mul(area[:], area[:], NMS_THR / (1.0 + NMS_THR))
            V.tensor_tensor(Mv, jview(area), iview(area), Alu.add)
            V.tensor_tensor(M[:], inter[:], M[:], Alu.is_gt)       # M flags

            # ---- stage 5: NMS via fixpoint iteration ----
            # M[p, a, b] = (IoU(a,b) > thr) & (b < a): k_{t+1}[a] =
            # valid[a] & ~any_b(M[a,b] & k_t[b]). The greedy keep set is the
            # unique fixpoint; 2 iterations cover suppression-chain depth 1
            # (measured depth on this data: 1) and the k4==k3 convergence
            # residual ships to the host, which falls back to an exact
            # reference recompute if it is ever nonzero.
            TRI = pool.tile([P, KN * KN], F32, tag="TRI")
            nc.sync.dma_start(TRI[:], bass.AP(tri_in, 0, [[0, P], [1, KN * KN]]))
            V.tensor_tensor(M[:], M[:], TRI[:], Alu.mult)
            valid = pool.tile([P, KN], F32, tag="valid")
            V.tensor_single_scalar(valid[:], vals[:, :KN], SCORE_THR, Alu.is_gt)
            ka = pool.tile([P, KN], F32, tag="ka")
            kb = pool.tile([P, KN], F32, tag="kb")
            supp = pool.tile([P, KN], F32, tag="supp")
            TMP, TMPv = big()
            k_prev, k_cur = None, valid
            for it in range(2):
                k_next = ka if it % 2 == 0 else kb
                V.tensor_tensor(
                    TMPv, Mv,
                    k_cur[:, None, :].broadcast_to([P, KN, KN]), Alu.mult)
                V.tensor_reduce(supp[:], TMPv, mybir.AxisListType.X, Alu.max)
                V.scalar_tensor_tensor(k_next[:], supp[:], 0.0, valid[:],
                                       Alu.is_equal, Alu.mult)
                k_prev, k_cur = k_cur, k_next
            dtile = pool.tile([P, KN], F32, tag="dtile")
            V.tensor_tensor(dtile[:], k_cur[:], k_prev[:], Alu.not_equal)
            dsum = pool.tile([P, 1], F32, tag="dsum")
            V.tensor_reduce(dsum[:], dtile[:], mybir.AxisListType.X, Alu.add)
            nc.sync.dma_start(ometa[:, 32:33], dsum[:])

            # ---- stage 6: masked scores + outputs ----
            good = k_cur
            pen = pool.tile([P, KN], F32, tag="pen")
            V.tensor_scalar(pen[:], good[:], -NEG, NEG, Alu.mult, Alu.add)  # 0 kept, NEG else
            V.tensor_tensor(good[:], good[:], vals[:, :KN], Alu.mult)
            V.tensor_tensor(good[:], good[:], pen[:], Alu.add)
            nc.sync.dma_start(omask[:], good[:])
            for kk, tl in enumerate((x1o, y1o, x2o, y2o)):
                nc.sync.dma_start(oboxes[kk], tl[:])
    if split_waits:
        _split_sync_waits(nc)
    return nc


# ---------------------------------------------------------------- host side

def _prep_core(all_rois, all_box_deltas, all_cls_scores, im_info, core):
    b, h = core // 2, core % 2
    sc = np.full((P, NPAD), NEG, np.float32)
    sc[:, :N] = all_cls_scores.reshape(B, N, C)[b, :, 1 + 40 * h:41 + 40 * h].T
    planes = np.zeros((NT, 8, 16, N), np.float32)
    planes[:, :, 0:4, :] = all_rois[b].T[None, None]
    dsl = all_box_deltas.reshape(B, N, C * 4)[b][:, 4 + 160 * h:164 + 160 * h]
    planes[:, :, 4:8, :] = dsl.T.reshape(NT, 8, 4, N)
    clip = np.empty((P, 2), np.float32)
    clip[:, 0] = np.float32(im_info[b, 1]) - np.float32(1.0)
    clip[:, 1] = np.float32(im_info[b, 0]) - np.float32(1.0)
    a = np.arange(KN)
    tri = (a[None, :] < a[:, None]).astype(np.float32).reshape(-1)
    return {"sc_in": np.ascontiguousarray(sc),
            "planes_in": np.ascontiguousarray(planes.reshape(NT, 128, N)),
            "clip_in": clip, "tri_in": tri}


def _merge(results):
    out_boxes = np.zeros((B, MAX_DET, 4), np.float32)
    out_scores = np.zeros((B, MAX_DET), np.float32)
    out_classes = np.zeros((B, MAX_DET), np.int32)
    ok_all = True
    for b in range(B):
        r0, r1 = results[2 * b], results[2 * b + 1]
        masked = np.concatenate([r0["omask"], r1["omask"]], axis=0)      # [80, K]
        boxes = np.concatenate([r0["oboxes"], r1["oboxes"]], axis=1)     # [4, 80, K]
        cert = max(float(r0["ometa"][:, KN].max()), float(r1["ometa"][:, KN].max()))
        if float(r0["ometa"][:, 32].sum()) != 0.0 or float(r1["ometa"][:, 32].sum()) != 0.0:
            ok_all = False
            break
        flat = masked.reshape(-1)
        order = np.argsort(-flat, kind="stable")[:MAX_DET]
        ssel = flat[order]
        if not (ssel[-1] > cert and ssel[-1] > -1.0e29):
            ok_all = False
            break
        cm_sel = order // KN
        out_scores[b] = ssel
        out_classes[b] = (cm_sel + 1).astype(np.int32)
        bt = boxes.reshape(4, -1)
        out_boxes[b] = bt[:, order].T
    if not ok_all:
        return None
    out_batch = np.repeat(np.arange(B, dtype=np.int32), MAX_DET)
    return (out_boxes.reshape(-1, 4), out_classes.reshape(-1),
            out_scores.reshape(-1), out_batch)


def kernel(all_rois, all_box_deltas, all_cls_scores, im_info, _sim=False, _trace=False):
    global _CACHED_NC
    all_rois = np.asarray(all_rois, np.float32)
    all_box_deltas = np.asarray(all_box_deltas, np.float32)
    all_cls_scores = np.asarray(all_cls_scores, np.float32)
    im_info = np.asarray(im_info, np.float32)

    if _sim:
        nc = build_program(split_waits=False)
    else:
        if _CACHED_NC is None:
            _CACHED_NC = build_program()
        nc = _CACHED_NC
    in_maps = [_prep_core(all_rois, all_box_deltas, all_cls_scores, im_info, c)
               for c in range(8)]

    if _sim:
        from concourse import bass_interp
        results = []
        for c in range(8):
            sim = bass_interp.CoreSim(nc)
            for k, v in in_maps[c].items():
                sim.tensor(k)[:] = v
            sim.simulate()
            results.append({k: np.array(sim.tensor(k))
                            for k in ("oboxes", "omask", "ometa", "oidx")})
        kr = None
    else:
        from concourse.bass_utils import run_bass_kernel_spmd
        kr = run_bass_kernel_spmd(nc, in_maps, list(range(8)), trace=_trace)
        results = kr.results

    merged = _merge(results)
    if merged is None:
        merged = _numpy_reference_fallback(all_rois, all_box_deltas,
                                           all_cls_scores, im_info)
    if _trace:
        return merged, kr
    return merged


# ---------------- exact numpy fallback (certification guard; never hit on
# ---------------- well-behaved inputs, kept for unconditional correctness)

def _numpy_reference_fallback(all_rois, all_box_deltas, all_cls_scores, im_info):
    deltas = all_box_deltas.reshape(B, N, C, 4)
    scores = all_cls_scores.reshape(B, N, C)
    sc = np.moveaxis(scores[:, :, 1:], 1, 2)
    valid = sc > SCORE_THR
    order = np.argsort(-np.where(valid, sc, -np.inf), axis=-1, kind="stable")
    out_boxes = np.zeros((B, MAX_DET, 4), np.float32)
    out_scores = np.zeros((B, MAX_DET), np.float32)
    out_classes = np.zeros((B, MAX_DET), np.int32)
    for b in range(B):
        Wm = np.float32(im_info[b, 1]) - 1.0
        Hm = np.float32(im_info[b, 0]) - 1.0
        cand_sc = np.full((Cm, N), -np.inf, np.float32)
        cand_bx = np.zeros((Cm, N, 4), np.float32)
        for cm in range(Cm):
            o = order[b, cm]
            r = all_rois[b, o]
            d = deltas[b, o, cm + 1]
            w = r[:, 2] - r[:, 0] + 1.0
            h = r[:, 3] - r[:, 1] + 1.0
            cx = r[:, 0] + 0.5 * w
            cy = r[:, 1] + 0.5 * h
            pcx = d[:, 0] / 10.0 * w + cx
            pcy = d[:, 1] / 10.0 * h + cy
            pw = np.exp(np.minimum(d[:, 2] / 5.0, np.float32(MAX_LOG_WH))) * w
            ph = np.exp(np.minimum(d[:, 3] / 5.0, np.float32(MAX_LOG_WH))) * h
            x1 = np.clip(pcx - 0.5 * pw, 0, Wm)
            y1 = np.clip(pcy - 0.5 * ph, 0, Hm)
            x2 = np.clip(pcx + 0.5 * pw - 1.0, 0, Wm)
            y2 = np.clip(pcy + 0.5 * ph - 1.0, 0, Hm)
            area = (x2 - x1 + 1.0) * (y2 - y1 + 1.0)
            s = sc[b, cm, o]
            keep = s > SCORE_THR
            for i in range(N):
                if not keep[i]:
                    continue
                xx1 = np.maximum(x1, x1[i]); yy1 = np.maximum(y1, y1[i])
                xx2 = np.minimum(x2, x2[i]); yy2 = np.minimum(y2, y2[i])
                inter = (np.maximum(xx2 - xx1 + 1.0, 0.0)
                         * np.maximum(yy2 - yy1 + 1.0, 0.0))
                iou = inter / (area + area[i] - inter)
                supp = (iou > NMS_THR) & (np.arange(N) > i) & keep
                keep &= ~supp
            cand_sc[cm, keep] = s[keep]
            cand_bx[cm] = np.stack([x1, y1, x2, y2], -1)
        flat = cand_sc.reshape(-1)
        o = np.argsort(-flat, kind="stable")[:MAX_DET]
        okm = np.isfinite(flat[o])
        out_scores[b] = np.where(okm, flat[o], 0.0)
        out_boxes[b] = np.where(okm[:, None], cand_bx.reshape(-1, 4)[o], 0.0)
        out_classes[b] = np.where(okm, o // N + 1, 0).astype(np.int32)
    out_batch = np.repeat(np.arange(B, dtype=np.int32), MAX_DET)
    return (out_boxes.reshape(-1, 4), out_classes.reshape(-1),
            out_scores.reshape(-1), out_batch)


# revision 42
# speedup vs baseline: 1.1391x; 1.0063x over previous
"""Trainium2 Bass kernel for nn_DetectionOutput (decode + per-class NMS + top-k).

Sharding: 8 cores = 4 images x 2 class-halves. Core c handles image b=c//2,
classes cm in [40h, 40h+40) where h=c%2 (cm = class-1, i.e. background dropped).

Algorithm (exact, certified): with uniform scores the per-image top-100 cutoff
is ~0.999 while the 25th-best score of any class is <=0.993, so only the top
KN=24 boxes per class can reach the output. Greedy-NMS keep of a sorted prefix
depends only on that prefix, so each core:
  1. 4 max8 rounds per class -> top-24 scores+indices, rank-24 cert value
     (DVE max8/find_index8/match_replace; ties resolve index-ascending,
     matching jnp stable argsort)
  2. per-round GPSIMD indirect_copy gathers of roi+delta planes for the 8
     freshly selected boxes, hidden under the next round's DVE work
  3. decode + clip boxes                            (DVE + one ACT exp)
  4. 24x24 IoU>0.7 strict-lower suppression matrix  (DVE broadcast ops)
  5. NMS keep = fixpoint of k <- valid & ~(M k), 3 iterations (covers
     suppression-chain depth 2; measured depth on this data is 1), with the
     k3==k2 residual shipped to the host
  6. emits kept-masked scores, boxes, cert + convergence meta
Host merges the two half-image candidate sets per image with the reference
tie-break (score desc, candidate index asc), certifies the prefix bound
(tau_100 > max cert, margin ~0.007 on uniform scores) and the fixpoint
residual; an exact numpy fallback runs if either check ever fails, so the
kernel is exact for any input.
"""
import sys

sys.path.insert(0, "/opt/trn_rl_repo")

import numpy as np
import concourse.bass as bass
import concourse.mybir as mybir
from concourse.tile import TileContext

F32 = mybir.dt.float32
U32 = mybir.dt.uint32
U16 = mybir.dt.uint16
Alu = mybir.AluOpType
Act = mybir.ActivationFunctionType

B, N, C = 4, 2000, 81
Cm = C - 1
P = 40            # class-problems per core
NPAD = 2048
K = 32            # scores extracted per class (4 max8 rounds)
KN = 24           # NMS prefix = gathered slots; cert score = rank KN
NROUND = K // 8       # top-K extracted; prefix = first KN, cert = vals[:, KN]
NT = 5            # gather tile-groups of 8 problems
GW = NT * KN      # gather scratch cols
MAX_DET = 100
SCORE_THR = 0.01
NMS_THR = 0.7
MAX_LOG_WH = float(np.log(1000.0 / 16.0))
NEG = -1.0e30

_CACHED_NC = None


def _patch_tile_tail_drain():
    """This walrus build rejects CTRL instructions carrying >2 sync waits
    (NCC_INLA001 'Too many sync wait commands' on the Tile tail drain).
    Emit sync-engine NOPs before the drain and spread the waits out, one
    per instruction."""
    import concourse.tile as tile_mod
    from concourse.vector_clock import ScopedClock

    if getattr(tile_mod.TileContext, "_tail_drain_patched", False):
        return

    def _drain_and_barrier(self, tick_clock, wait_clock):
        nc = self.nc
        nops = [nc.sync.nop(nofuse=True) for _ in range(30)]
        drain_inst = nc.sync.drain()
        wait_clock.add_sem_waits(
            drain_inst.ins, ScopedClock({None: tick_clock.global_clock})
        )
        waits = list(drain_inst.ins.sync_info.on_wait or [])
        if len(waits) > 1:
            assert len(waits) <= len(nops) + 1
            drain_inst.ins.sync_info.on_wait = [waits[0]]
            for w, nop in zip(waits[1:], nops):
                nop.ins.sync_info = mybir.SyncInfo(on_wait=[w], on_update=[])
        nc.all_engine_barrier()
        assert self.sems is not None
        popped = nc._tile_sem_poison_stack.pop()
        assert popped is self._sem_poison
        nc.clear_and_free_semaphores(list(self.sems.allocated().values()))
        nc.all_engine_barrier()

    tile_mod.TileContext._drain_and_barrier = _drain_and_barrier
    tile_mod.TileContext._tail_drain_patched = True


def _split_sync_waits(nc, max_waits=1):
    """Walrus codegen in this container rejects instructions carrying more
    than 1-2 sync waits (class-dependent). Cap every instruction at
    `max_waits` by hoisting the excess onto same-engine NoOps inserted
    immediately before it (engine blocks at the same program point, so
    semantics and Tile's schedule-order guarantees are preserved)."""
    import bass_rust
    ctr = 0
    for f in nc.m.functions:
        for bb in f.blocks:
            ins_list = bb.instructions
            new = []
            for inst in ins_list:
                si = inst.sync_info
                waits = list(si.on_wait) if si and si.on_wait else []
                if len(waits) > max_waits:
                    for w in waits[max_waits:]:
                        ctr += 1
                        nop = bass_rust.InstNoOp(name=f"WSPLIT-{ctr}")
                        nop.engine = inst.engine
                        nop.sync_info = mybir.SyncInfo(on_wait=[w], on_update=[])
                        new.append(nop)
                    inst.sync_info = mybir.SyncInfo(
                        on_wait=waits[:max_waits],
                        on_update=list(si.on_update) if si.on_update else [])
                new.append(inst)
            ins_list[:] = new
    return ctr


def build_program(split_waits=True):
    _patch_tile_tail_drain()
    nc = bass.Bass()
    sc_in = nc.dram_tensor("sc_in", [P, NPAD], F32, kind="ExternalInput")
    planes_in = nc.dram_tensor("planes_in", [NT, 128, N], F32, kind="ExternalInput")
    clip_in = nc.dram_tensor("clip_in", [P, 2], F32, kind="ExternalInput")  # [Wm, Hm]
    oboxes = nc.dram_tensor("oboxes", [4, P, KN], F32, kind="ExternalOutput")
    omask = nc.dram_tensor("omask", [P, KN], F32, kind="ExternalOutput")
    ometa = nc.dram_tensor("ometa", [P, 40], F32, kind="ExternalOutput")
    tri_in = nc.dram_tensor("tri_in", [KN * KN], F32, kind="ExternalInput")
    scr_gat = nc.dram_tensor("scr_gat", [128 * GW], F32)

    with TileContext(nc) as tc:
        with (
            tc.tile_pool(name="pool", bufs=1) as pool,
            tc.tile_pool(name="plpool", bufs=1) as plpool,
        ):
            sc = pool.tile([P, NPAD], F32)
            nc.sync.dma_start(sc[:], sc_in[:])
            clip = pool.tile([P, 2], F32)
            nc.sync.dma_start(clip[:], clip_in[:])

            # plane tiles up-front; their DMAs overlap the topk rounds
            pls = []
            for t in range(NT):
                pl = plpool.tile([128, N], F32, tag=f"pl{t}", name=f"pl{t}")
                nc.sync.dma_start(pl[:], planes_in[t])
                pls.append(pl)

            # ---- stage 1+2: topk rounds with per-round gather ----
            # idxr row p holds round r's 8 indices at cols 16r..16r+8 (u16);
            # iwt_rt[16g+j, 0] = idxr[8t+g, 16r+j] so indirect_copy's "(s p)"
            # unwrap yields slots 8r..8r+7 in order. Gathers for round r run
            # on GPSIMD under round r+1's DVE work.
            vals = pool.tile([P, NROUND * 8], F32)
            idxr = pool.tile([P, 16 * NROUND], U16)
            nc.vector.memset(idxr[:], 0)
            gat = pool.tile([128, NT * KN], F32)
            irap = idxr[:]
            pk = []
            for k in range(8):
                pkt = pool.tile([P, KN], F32, tag=f"pk{k}", name=f"pk{k}")
                pk.append(pkt)
            for r in range(NROUND):
                s8 = slice(r * 8, (r + 1) * 8)
                nc.vector.max(vals[:, s8], sc[:])
                if r * 8 >= KN:
                    continue    # cert-only round: no indices, no gather
                nc.vector.max_index(idxr[:, 16 * r:16 * r + 8], vals[:, s8], sc[:])
                if r + 1 < NROUND:
                    nc.vector.match_replace(sc[:], vals[:, s8], sc[:], NEG)
                for t in range(NT):
                    iwt = plpool.tile([128, 1], U16, tag=f"iw{t}_{r}",
                                      name=f"iw{t}_{r}")
                    nc.sync.dma_start(
                        iwt[:], bass.AP(irap.tensor,
                                        irap.offset + 8 * t * 16 * NROUND + 16 * r,
                                        [[16 * NROUND, 8], [1, 16], [1, 1]]))
                    nc.gpsimd.indirect_copy(
                        gat[:, t * KN + 8 * r:t * KN + 8 * r + 8].rearrange(
                            "p (i one) -> p i one", one=1),
                        pls[t][:], iwt[:], True)
                # bounce this round's gathered columns to DRAM for un-interleave
                gap = gat[:]
                nc.sync.dma_start(
                    bass.AP(scr_gat, 8 * r, [[GW, 128], [KN, NT], [1, 8]]),
                    bass.AP(gap.tensor, gap.offset + 8 * r,
                            [[GW, 128], [KN, NT], [1, 8]]))
            nc.sync.dma_start(ometa[:, 0:NROUND * 8], vals[:])
            for k in range(8):
                nc.sync.dma_start(
                    pk[k][:], bass.AP(scr_gat, k * GW, [[KN, NT], [16 * GW, 8], [1, KN]]))
            x1r, y1r, x2r, y2r, dx, dy, dw, dh = pk

            _tagn = [0]

            def tile():
                _tagn[0] += 1
                return pool.tile([P, KN], F32, tag=f"dec{_tagn[0]}", name=f"dec{_tagn[0]}")

            V = nc.vector
            # ---- stage 3: decode (mirrors reference fp op order) ----
            w = pool.tile([P, KN], F32, tag="w")
            h = pool.tile([P, KN], F32, tag="h")
            cx, cy, pcx, pcy, pw, ph, area = (tile() for _ in range(7))
            ew = pool.tile([P, 2 * KN], F32, tag="ew")
            aw = pool.tile([P, KN], F32, tag="aw")
            x1o, y1o, x2o, y2o = x1r, y1r, x2r, y2r

            def decode_wave(sl):
                s2 = slice(KN + sl.start, KN + sl.stop)
                V.tensor_tensor(w[:, sl], x2r[:, sl], x1r[:, sl], Alu.subtract)
                V.tensor_scalar_add(w[:, sl], w[:, sl], 1.0)
                V.tensor_tensor(h[:, sl], y2r[:, sl], y1r[:, sl], Alu.subtract)
                V.tensor_scalar_add(h[:, sl], h[:, sl], 1.0)
                V.tensor_scalar_mul(cx[:, sl], w[:, sl], 0.5)
                V.tensor_tensor(cx[:, sl], x1r[:, sl], cx[:, sl], Alu.add)
                V.tensor_scalar_mul(cy[:, sl], h[:, sl], 0.5)
                V.tensor_tensor(cy[:, sl], y1r[:, sl], cy[:, sl], Alu.add)
                V.tensor_scalar_mul(pcx[:, sl], dx[:, sl], 0.1)
                V.tensor_tensor(pcx[:, sl], pcx[:, sl], w[:, sl], Alu.mult)
                V.tensor_tensor(pcx[:, sl], pcx[:, sl], cx[:, sl], Alu.add)
                V.tensor_scalar_mul(pcy[:, sl], dy[:, sl], 0.1)
                V.tensor_tensor(pcy[:, sl], pcy[:, sl], h[:, sl], Alu.mult)
                V.tensor_tensor(pcy[:, sl], pcy[:, sl], cy[:, sl], Alu.add)
                V.tensor_scalar(ew[:, sl], dw[:, sl], 0.2, MAX_LOG_WH,
                                Alu.mult, Alu.min)
                V.tensor_scalar(ew[:, s2], dh[:, sl], 0.2, MAX_LOG_WH,
                                Alu.mult, Alu.min)
                nc.scalar.activation(ew[:, sl], ew[:, sl], Act.Exp)
                nc.scalar.activation(ew[:, s2], ew[:, s2], Act.Exp)
                V.tensor_tensor(pw[:, sl], ew[:, sl], w[:, sl], Alu.mult)
                V.tensor_tensor(ph[:, sl], ew[:, s2], h[:, sl], Alu.mult)
                V.tensor_scalar_mul(pw[:, sl], pw[:, sl], 0.5)
                V.tensor_tensor(x1o[:, sl], pcx[:, sl], pw[:, sl], Alu.subtract)
                V.tensor_tensor(x2o[:, sl], pcx[:, sl], pw[:, sl], Alu.add)
                V.tensor_scalar_add(x2o[:, sl], x2o[:, sl], -1.0)
                V.tensor_scalar_mul(ph[:, sl], ph[:, sl], 0.5)
                V.tensor_tensor(y1o[:, sl], pcy[:, sl], ph[:, sl], Alu.subtract)
                V.tensor_tensor(y2o[:, sl], pcy[:, sl], ph[:, sl], Alu.add)
                V.tensor_scalar_add(y2o[:, sl], y2o[:, sl], -1.0)
                for tl, cc in ((x1o, 0), (x2o, 0), (y1o, 1), (y2o, 1)):
                    V.tensor_scalar(tl[:, sl], tl[:, sl], 0.0,
                                    clip[:, cc:cc + 1], Alu.max, Alu.min)
                V.tensor_tensor(aw[:, sl], x2o[:, sl], x1o[:, sl], Alu.subtract)
                V.tensor_scalar_add(aw[:, sl], aw[:, sl], 1.0)
                V.tensor_tensor(area[:, sl], y2o[:, sl], y1o[:, sl], Alu.subtract)
                V.tensor_scalar_add(area[:, sl], area[:, sl], 1.0)
                V.tensor_tensor(area[:, sl], area[:, sl], aw[:, sl], Alu.mult)

            decode_wave(slice(0, KN))

            # ---- stage 4: suppression matrix M[p, i, j] = IoU(i,j) > thr ----
            def iview(t):
                return t[:, :, None].broadcast_to([P, KN, KN])

            def jview(t):
                return t[:, None, :].broadcast_to([P, KN, KN])

            def big():
                _tagn[0] += 1
                tl = pool.tile([P, KN * KN], F32, tag=f"big{_tagn[0]}", name=f"big{_tagn[0]}")
                return tl, tl[:].rearrange("p (i j) -> p i j", i=KN)

            M, Mv = big()
            w_t, w_v = big()
            xx1, xx1v = big()
            V.tensor_tensor(xx1v, iview(x1o), jview(x1o), Alu.max)
            V.tensor_tensor(Mv, iview(x2o), jview(x2o), Alu.min)
            V.tensor_tensor(w_v, Mv, xx1v, Alu.subtract)
            V.tensor_scalar(w_t[:], w_t[:], 1.0, 0.0, Alu.add, Alu.max)
            V.tensor_tensor(xx1v, iview(y1o), jview(y1o), Alu.max)
            V.tensor_tensor(Mv, iview(y2o), jview(y2o), Alu.min)
            V.tensor_tensor(xx1v, Mv, xx1v, Alu.subtract)
            V.tensor_scalar(xx1[:], xx1[:], 1.0, 0.0, Alu.add, Alu.max)
            inter = w_t
            V.tensor_tensor(inter[:], w_t[:], xx1[:], Alu.mult)   # inter
            # IoU > t  <=>  inter > (t/(1+t))*(Ai+Aj); margin |IoU-0.7| on
            # this data is 2e-5 >> the ~1e-7 fp discrepancy of the rewrite
            V.tensor_scalar_mul(area[:], area[:], NMS_THR / (1.0 + NMS_THR))
            V.tensor_tensor(Mv, jview(area), iview(area), Alu.add)
            V.tensor_tensor(M[:], inter[:], M[:], Alu.is_gt)       # M flags

            # ---- stage 5: NMS via fixpoint iteration ----
            # M[p, a, b] = (IoU(a,b) > thr) & (b < a): k_{t+1}[a] =
            # valid[a] & ~any_b(M[a,b] & k_t[b]). The greedy keep set is the
            # unique fixpoint; 2 iterations cover suppression-chain depth 1
            # (measured depth on this data: 1) and the k4==k3 convergence
            # residual ships to the host, which falls back to an exact
            # reference recompute if it is ever nonzero.
            TRI = pool.tile([P, KN * KN], F32, tag="TRI")
            nc.sync.dma_start(TRI[:], bass.AP(tri_in, 0, [[0, P], [1, KN * KN]]))
            V.tensor_tensor(M[:], M[:], TRI[:], Alu.mult)
            valid = pool.tile([P, KN], F32, tag="valid")
            V.tensor_single_scalar(valid[:], vals[:, :KN], SCORE_THR, Alu.is_gt)
            ka = pool.tile([P, KN], F32, tag="ka")
            kb = pool.tile([P, KN], F32, tag="kb")
            supp = pool.tile([P, KN], F32, tag="supp")
            TMP, TMPv = big()
            k_prev, k_cur = None, valid
            for it in range(2):
                k_next = ka if it % 2 == 0 else kb
                V.tensor_tensor(
                    TMPv, Mv,
                    k_cur[:, None, :].broadcast_to([P, KN, KN]), Alu.mult)
                V.tensor_reduce(supp[:], TMPv, mybir.AxisListType.X, Alu.max)
                V.scalar_tensor_tensor(k_next[:], supp[:], 0.0, valid[:],
                                       Alu.is_equal, Alu.mult)
                k_prev, k_cur = k_cur, k_next
            dtile = pool.tile([P, KN], F32, tag="dtile")
            V.tensor_tensor(dtile[:], k_cur[:], k_prev[:], Alu.not_equal)
            dsum = pool.tile([P, 1], F32, tag="dsum")
            V.tensor_reduce(dsum[:], dtile[:], mybir.AxisListType.X, Alu.add)
            nc.sync.dma_start(ometa[:, 32:33], dsum[:])

            # ---- stage 6: masked scores + outputs ----
            good = k_cur
            pen = pool.tile([P, KN], F32, tag="pen")
            V.tensor_scalar(pen[:], good[:], -NEG, NEG, Alu.mult, Alu.add)  # 0 kept, NEG else
            V.tensor_tensor(good[:], good[:], vals[:, :KN], Alu.mult)
            V.tensor_tensor(good[:], good[:], pen[:], Alu.add)
            nc.sync.dma_start(omask[:], good[:])
            for kk, tl in enumerate((x1o, y1o, x2o, y2o)):
                nc.sync.dma_start(oboxes[kk], tl[:])
    if split_waits:
        _split_sync_waits(nc)
    return nc


# ---------------------------------------------------------------- host side

def _prep_core(all_rois, all_box_deltas, all_cls_scores, im_info, core):
    b, h = core // 2, core % 2
    sc = np.full((P, NPAD), NEG, np.float32)
    sc[:, :N] = all_cls_scores.reshape(B, N, C)[b, :, 1 + 40 * h:41 + 40 * h].T
    planes = np.zeros((NT, 8, 16, N), np.float32)
    planes[:, :, 0:4, :] = all_rois[b].T[None, None]
    dsl = all_box_deltas.reshape(B, N, C * 4)[b][:, 4 + 160 * h:164 + 160 * h]
    planes[:, :, 4:8, :] = dsl.T.reshape(NT, 8, 4, N)
    clip = np.empty((P, 2), np.float32)
    clip[:, 0] = np.float32(im_info[b, 1]) - np.float32(1.0)
    clip[:, 1] = np.float32(im_info[b, 0]) - np.float32(1.0)
    a = np.arange(KN)
    tri = (a[None, :] < a[:, None]).astype(np.float32).reshape(-1)
    return {"sc_in": np.ascontiguousarray(sc),
            "planes_in": np.ascontiguousarray(planes.reshape(NT, 128, N)),
            "clip_in": clip, "tri_in": tri}


def _merge(results):
    out_boxes = np.zeros((B, MAX_DET, 4), np.float32)
    out_scores = np.zeros((B, MAX_DET), np.float32)
    out_classes = np.zeros((B, MAX_DET), np.int32)
    ok_all = True
    for b in range(B):
        r0, r1 = results[2 * b], results[2 * b + 1]
        masked = np.concatenate([r0["omask"], r1["omask"]], axis=0)      # [80, K]
        boxes = np.concatenate([r0["oboxes"], r1["oboxes"]], axis=1)     # [4, 80, K]
        cert = max(float(r0["ometa"][:, KN].max()), float(r1["ometa"][:, KN].max()))
        if float(r0["ometa"][:, 32].sum()) != 0.0 or float(r1["ometa"][:, 32].sum()) != 0.0:
            ok_all = False
            break
        flat = masked.reshape(-1)
        order = np.argsort(-flat, kind="stable")[:MAX_DET]
        ssel = flat[order]
        if not (ssel[-1] > cert and ssel[-1] > -1.0e29):
            ok_all = False
            break
        cm_sel = order // KN
        out_scores[b] = ssel
        out_classes[b] = (cm_sel + 1).astype(np.int32)
        bt = boxes.reshape(4, -1)
        out_boxes[b] = bt[:, order].T
    if not ok_all:
        return None
    out_batch = np.repeat(np.arange(B, dtype=np.int32), MAX_DET)
    return (out_boxes.reshape(-1, 4), out_classes.reshape(-1),
            out_scores.reshape(-1), out_batch)


def kernel(all_rois, all_box_deltas, all_cls_scores, im_info, _sim=False, _trace=False):
    global _CACHED_NC
    all_rois = np.asarray(all_rois, np.float32)
    all_box_deltas = np.asarray(all_box_deltas, np.float32)
    all_cls_scores = np.asarray(all_cls_scores, np.float32)
    im_info = np.asarray(im_info, np.float32)

    if _sim:
        nc = build_program(split_waits=False)
    else:
        if _CACHED_NC is None:
            _CACHED_NC = build_program()
        nc = _CACHED_NC
    in_maps = [_prep_core(all_rois, all_box_deltas, all_cls_scores, im_info, c)
               for c in range(8)]

    if _sim:
        from concourse import bass_interp
        results = []
        for c in range(8):
            sim = bass_interp.CoreSim(nc)
            for k, v in in_maps[c].items():
                sim.tensor(k)[:] = v
            sim.simulate()
            results.append({k: np.array(sim.tensor(k))
                            for k in ("oboxes", "omask", "ometa")})
        kr = None
    else:
        from concourse.bass_utils import run_bass_kernel_spmd
        kr = run_bass_kernel_spmd(nc, in_maps, list(range(8)), trace=_trace)
        results = kr.results

    merged = _merge(results)
    if merged is None:
        merged = _numpy_reference_fallback(all_rois, all_box_deltas,
                                           all_cls_scores, im_info)
    if _trace:
        return merged, kr
    return merged


# ---------------- exact numpy fallback (certification guard; never hit on
# ---------------- well-behaved inputs, kept for unconditional correctness)

def _numpy_reference_fallback(all_rois, all_box_deltas, all_cls_scores, im_info):
    deltas = all_box_deltas.reshape(B, N, C, 4)
    scores = all_cls_scores.reshape(B, N, C)
    sc = np.moveaxis(scores[:, :, 1:], 1, 2)
    valid = sc > SCORE_THR
    order = np.argsort(-np.where(valid, sc, -np.inf), axis=-1, kind="stable")
    out_boxes = np.zeros((B, MAX_DET, 4), np.float32)
    out_scores = np.zeros((B, MAX_DET), np.float32)
    out_classes = np.zeros((B, MAX_DET), np.int32)
    for b in range(B):
        Wm = np.float32(im_info[b, 1]) - 1.0
        Hm = np.float32(im_info[b, 0]) - 1.0
        cand_sc = np.full((Cm, N), -np.inf, np.float32)
        cand_bx = np.zeros((Cm, N, 4), np.float32)
        for cm in range(Cm):
            o = order[b, cm]
            r = all_rois[b, o]
            d = deltas[b, o, cm + 1]
            w = r[:, 2] - r[:, 0] + 1.0
            h = r[:, 3] - r[:, 1] + 1.0
            cx = r[:, 0] + 0.5 * w
            cy = r[:, 1] + 0.5 * h
            pcx = d[:, 0] / 10.0 * w + cx
            pcy = d[:, 1] / 10.0 * h + cy
            pw = np.exp(np.minimum(d[:, 2] / 5.0, np.float32(MAX_LOG_WH))) * w
            ph = np.exp(np.minimum(d[:, 3] / 5.0, np.float32(MAX_LOG_WH))) * h
            x1 = np.clip(pcx - 0.5 * pw, 0, Wm)
            y1 = np.clip(pcy - 0.5 * ph, 0, Hm)
            x2 = np.clip(pcx + 0.5 * pw - 1.0, 0, Wm)
            y2 = np.clip(pcy + 0.5 * ph - 1.0, 0, Hm)
            area = (x2 - x1 + 1.0) * (y2 - y1 + 1.0)
            s = sc[b, cm, o]
            keep = s > SCORE_THR
            for i in range(N):
                if not keep[i]:
                    continue
                xx1 = np.maximum(x1, x1[i]); yy1 = np.maximum(y1, y1[i])
                xx2 = np.minimum(x2, x2[i]); yy2 = np.minimum(y2, y2[i])
                inter = (np.maximum(xx2 - xx1 + 1.0, 0.0)
                         * np.maximum(yy2 - yy1 + 1.0, 0.0))
                iou = inter / (area + area[i] - inter)
                supp = (iou > NMS_THR) & (np.arange(N) > i) & keep
                keep &= ~supp
            cand_sc[cm, keep] = s[keep]
            cand_bx[cm] = np.stack([x1, y1, x2, y2], -1)
        flat = cand_sc.reshape(-1)
        o = np.argsort(-flat, kind="stable")[:MAX_DET]
        okm = np.isfinite(flat[o])
        out_scores[b] = np.where(okm, flat[o], 0.0)
        out_boxes[b] = np.where(okm[:, None], cand_bx.reshape(-1, 4)[o], 0.0)
        out_classes[b] = np.where(okm, o // N + 1, 0).astype(np.int32)
    out_batch = np.repeat(np.arange(B, dtype=np.int32), MAX_DET)
    return (out_boxes.reshape(-1, 4), out_classes.reshape(-1),
            out_scores.reshape(-1), out_batch)


# revision 43
# speedup vs baseline: 1.1666x; 1.0241x over previous
"""Trainium2 Bass kernel for nn_DetectionOutput (decode + per-class NMS + top-k).

Sharding: 8 cores = 4 images x 2 class-halves. Core c handles image b=c//2,
classes cm in [40h, 40h+40) where h=c%2 (cm = class-1, i.e. background dropped).

Algorithm (exact, certified): with uniform scores the per-image top-100 cutoff
is ~0.999 while the 25th-best score of any class is <=0.993, so only the top
KN=24 boxes per class can reach the output. Greedy-NMS keep of a sorted prefix
depends only on that prefix, so each core:
  1. 4 max8 rounds per class -> top-24 scores+indices, rank-24 cert value
     (DVE max8/find_index8/match_replace; ties resolve index-ascending,
     matching jnp stable argsort)
  2. per-round GPSIMD indirect_copy gathers of roi+delta planes for the 8
     freshly selected boxes, hidden under the next round's DVE work
  3. decode + clip boxes                            (DVE + one ACT exp)
  4. 24x24 IoU>0.7 strict-lower suppression matrix  (DVE broadcast ops)
  5. NMS keep = fixpoint of k <- valid & ~(M k), 3 iterations (covers
     suppression-chain depth 2; measured depth on this data is 1), with the
     k3==k2 residual shipped to the host
  6. emits kept-masked scores, boxes, cert + convergence meta
Host merges the two half-image candidate sets per image with the reference
tie-break (score desc, candidate index asc), certifies the prefix bound
(tau_100 > max cert, margin ~0.007 on uniform scores) and the fixpoint
residual; an exact numpy fallback runs if either check ever fails, so the
kernel is exact for any input.
"""
import sys

sys.path.insert(0, "/opt/trn_rl_repo")

import numpy as np
import concourse.bass as bass
import concourse.mybir as mybir
from concourse.tile import TileContext

F32 = mybir.dt.float32
U32 = mybir.dt.uint32
U16 = mybir.dt.uint16
Alu = mybir.AluOpType
Act = mybir.ActivationFunctionType

B, N, C = 4, 2000, 81
Cm = C - 1
P = 40            # class-problems per core
NPAD = 2048
K = 32            # scores extracted per class (4 max8 rounds)
KN = 24           # NMS prefix = gathered slots; cert score = rank KN
NROUND = KN // 8      # 3 rounds; cert = vals[:, KN-1] (upper-bounds all unexamined)
NT = 5            # gather tile-groups of 8 problems
GW = NT * KN      # gather scratch cols
MAX_DET = 100
SCORE_THR = 0.01
NMS_THR = 0.7
MAX_LOG_WH = float(np.log(1000.0 / 16.0))
NEG = -1.0e30

_CACHED_NC = None


def _patch_tile_tail_drain():
    """This walrus build rejects CTRL instructions carrying >2 sync waits
    (NCC_INLA001 'Too many sync wait commands' on the Tile tail drain).
    Emit sync-engine NOPs before the drain and spread the waits out, one
    per instruction."""
    import concourse.tile as tile_mod
    from concourse.vector_clock import ScopedClock

    if getattr(tile_mod.TileContext, "_tail_drain_patched", False):
        return

    def _drain_and_barrier(self, tick_clock, wait_clock):
        nc = self.nc
        nops = [nc.sync.nop(nofuse=True) for _ in range(30)]
        drain_inst = nc.sync.drain()
        wait_clock.add_sem_waits(
            drain_inst.ins, ScopedClock({None: tick_clock.global_clock})
        )
        waits = list(drain_inst.ins.sync_info.on_wait or [])
        if len(waits) > 1:
            assert len(waits) <= len(nops) + 1
            drain_inst.ins.sync_info.on_wait = [waits[0]]
            for w, nop in zip(waits[1:], nops):
                nop.ins.sync_info = mybir.SyncInfo(on_wait=[w], on_update=[])
        nc.all_engine_barrier()
        assert self.sems is not None
        popped = nc._tile_sem_poison_stack.pop()
        assert popped is self._sem_poison
        nc.clear_and_free_semaphores(list(self.sems.allocated().values()))
        nc.all_engine_barrier()

    tile_mod.TileContext._drain_and_barrier = _drain_and_barrier
    tile_mod.TileContext._tail_drain_patched = True


def _split_sync_waits(nc, max_waits=1):
    """Walrus codegen in this container rejects instructions carrying more
    than 1-2 sync waits (class-dependent). Cap every instruction at
    `max_waits` by hoisting the excess onto same-engine NoOps inserted
    immediately before it (engine blocks at the same program point, so
    semantics and Tile's schedule-order guarantees are preserved)."""
    import bass_rust
    ctr = 0
    for f in nc.m.functions:
        for bb in f.blocks:
            ins_list = bb.instructions
            new = []
            for inst in ins_list:
                si = inst.sync_info
                waits = list(si.on_wait) if si and si.on_wait else []
                if len(waits) > max_waits:
                    for w in waits[max_waits:]:
                        ctr += 1
                        nop = bass_rust.InstNoOp(name=f"WSPLIT-{ctr}")
                        nop.engine = inst.engine
                        nop.sync_info = mybir.SyncInfo(on_wait=[w], on_update=[])
                        new.append(nop)
                    inst.sync_info = mybir.SyncInfo(
                        on_wait=waits[:max_waits],
                        on_update=list(si.on_update) if si.on_update else [])
                new.append(inst)
            ins_list[:] = new
    return ctr


def build_program(split_waits=True):
    _patch_tile_tail_drain()
    nc = bass.Bass()
    sc_in = nc.dram_tensor("sc_in", [P, NPAD], F32, kind="ExternalInput")
    planes_in = nc.dram_tensor("planes_in", [NT, 128, N], F32, kind="ExternalInput")
    clip_in = nc.dram_tensor("clip_in", [P, 2], F32, kind="ExternalInput")  # [Wm, Hm]
    oboxes = nc.dram_tensor("oboxes", [4, P, KN], F32, kind="ExternalOutput")
    omask = nc.dram_tensor("omask", [P, KN], F32, kind="ExternalOutput")
    ometa = nc.dram_tensor("ometa", [P, 40], F32, kind="ExternalOutput")
    tri_in = nc.dram_tensor("tri_in", [KN * KN], F32, kind="ExternalInput")
    scr_gat = nc.dram_tensor("scr_gat", [128 * GW], F32)

    with TileContext(nc) as tc:
        with (
            tc.tile_pool(name="pool", bufs=1) as pool,
            tc.tile_pool(name="plpool", bufs=1) as plpool,
        ):
            sc = pool.tile([P, NPAD], F32)
            nc.sync.dma_start(sc[:], sc_in[:])
            clip = pool.tile([P, 2], F32)
            nc.sync.dma_start(clip[:], clip_in[:])

            # plane tiles up-front; their DMAs overlap the topk rounds
            pls = []
            for t in range(NT):
                pl = plpool.tile([128, N], F32, tag=f"pl{t}", name=f"pl{t}")
                nc.sync.dma_start(pl[:], planes_in[t])
                pls.append(pl)

            # ---- stage 1+2: topk rounds with per-round gather ----
            # idxr row p holds round r's 8 indices at cols 16r..16r+8 (u16);
            # iwt_rt[16g+j, 0] = idxr[8t+g, 16r+j] so indirect_copy's "(s p)"
            # unwrap yields slots 8r..8r+7 in order. Gathers for round r run
            # on GPSIMD under round r+1's DVE work.
            vals = pool.tile([P, NROUND * 8], F32)
            idxr = pool.tile([P, 16 * NROUND], U16)
            nc.vector.memset(idxr[:], 0)
            gat = pool.tile([128, NT * KN], F32)
            irap = idxr[:]
            pk = []
            for k in range(8):
                pkt = pool.tile([P, KN], F32, tag=f"pk{k}", name=f"pk{k}")
                pk.append(pkt)
            for r in range(NROUND):
                s8 = slice(r * 8, (r + 1) * 8)
                nc.vector.max(vals[:, s8], sc[:])
                if r * 8 >= KN:
                    continue    # cert-only round: no indices, no gather
                nc.vector.max_index(idxr[:, 16 * r:16 * r + 8], vals[:, s8], sc[:])
                if r + 1 < NROUND:
                    nc.vector.match_replace(sc[:], vals[:, s8], sc[:], NEG)
                for t in range(NT):
                    iwt = plpool.tile([128, 1], U16, tag=f"iw{t}_{r}",
                                      name=f"iw{t}_{r}")
                    nc.sync.dma_start(
                        iwt[:], bass.AP(irap.tensor,
                                        irap.offset + 8 * t * 16 * NROUND + 16 * r,
                                        [[16 * NROUND, 8], [1, 16], [1, 1]]))
                    nc.gpsimd.indirect_copy(
                        gat[:, t * KN + 8 * r:t * KN + 8 * r + 8].rearrange(
                            "p (i one) -> p i one", one=1),
                        pls[t][:], iwt[:], True)
                # bounce this round's gathered columns to DRAM for un-interleave
                gap = gat[:]
                nc.sync.dma_start(
                    bass.AP(scr_gat, 8 * r, [[GW, 128], [KN, NT], [1, 8]]),
                    bass.AP(gap.tensor, gap.offset + 8 * r,
                            [[GW, 128], [KN, NT], [1, 8]]))
            nc.sync.dma_start(ometa[:, 0:NROUND * 8], vals[:])
            for k in range(8):
                nc.sync.dma_start(
                    pk[k][:], bass.AP(scr_gat, k * GW, [[KN, NT], [16 * GW, 8], [1, KN]]))
            x1r, y1r, x2r, y2r, dx, dy, dw, dh = pk

            _tagn = [0]

            def tile():
                _tagn[0] += 1
                return pool.tile([P, KN], F32, tag=f"dec{_tagn[0]}", name=f"dec{_tagn[0]}")

            V = nc.vector
            # ---- stage 3: decode (mirrors reference fp op order) ----
            w = pool.tile([P, KN], F32, tag="w")
            h = pool.tile([P, KN], F32, tag="h")
            cx, cy, pcx, pcy, pw, ph, area = (tile() for _ in range(7))
            ew = pool.tile([P, 2 * KN], F32, tag="ew")
            aw = pool.tile([P, KN], F32, tag="aw")
            x1o, y1o, x2o, y2o = x1r, y1r, x2r, y2r

            def decode_wave(sl):
                s2 = slice(KN + sl.start, KN + sl.stop)
                V.tensor_tensor(w[:, sl], x2r[:, sl], x1r[:, sl], Alu.subtract)
                V.tensor_scalar_add(w[:, sl], w[:, sl], 1.0)
                V.tensor_tensor(h[:, sl], y2r[:, sl], y1r[:, sl], Alu.subtract)
                V.tensor_scalar_add(h[:, sl], h[:, sl], 1.0)
                V.tensor_scalar_mul(cx[:, sl], w[:, sl], 0.5)
                V.tensor_tensor(cx[:, sl], x1r[:, sl], cx[:, sl], Alu.add)
                V.tensor_scalar_mul(cy[:, sl], h[:, sl], 0.5)
                V.tensor_tensor(cy[:, sl], y1r[:, sl], cy[:, sl], Alu.add)
                V.tensor_scalar_mul(pcx[:, sl], dx[:, sl], 0.1)
                V.tensor_tensor(pcx[:, sl], pcx[:, sl], w[:, sl], Alu.mult)
                V.tensor_tensor(pcx[:, sl], pcx[:, sl], cx[:, sl], Alu.add)
                V.tensor_scalar_mul(pcy[:, sl], dy[:, sl], 0.1)
                V.tensor_tensor(pcy[:, sl], pcy[:, sl], h[:, sl], Alu.mult)
                V.tensor_tensor(pcy[:, sl], pcy[:, sl], cy[:, sl], Alu.add)
                V.tensor_scalar(ew[:, sl], dw[:, sl], 0.2, MAX_LOG_WH,
                                Alu.mult, Alu.min)
                V.tensor_scalar(ew[:, s2], dh[:, sl], 0.2, MAX_LOG_WH,
                                Alu.mult, Alu.min)
                nc.scalar.activation(ew[:, sl], ew[:, sl], Act.Exp)
                nc.scalar.activation(ew[:, s2], ew[:, s2], Act.Exp)
                V.tensor_tensor(pw[:, sl], ew[:, sl], w[:, sl], Alu.mult)
                V.tensor_tensor(ph[:, sl], ew[:, s2], h[:, sl], Alu.mult)
                V.tensor_scalar_mul(pw[:, sl], pw[:, sl], 0.5)
                V.tensor_tensor(x1o[:, sl], pcx[:, sl], pw[:, sl], Alu.subtract)
                V.tensor_tensor(x2o[:, sl], pcx[:, sl], pw[:, sl], Alu.add)
                V.tensor_scalar_add(x2o[:, sl], x2o[:, sl], -1.0)
                V.tensor_scalar_mul(ph[:, sl], ph[:, sl], 0.5)
                V.tensor_tensor(y1o[:, sl], pcy[:, sl], ph[:, sl], Alu.subtract)
                V.tensor_tensor(y2o[:, sl], pcy[:, sl], ph[:, sl], Alu.add)
                V.tensor_scalar_add(y2o[:, sl], y2o[:, sl], -1.0)
                for tl, cc in ((x1o, 0), (x2o, 0), (y1o, 1), (y2o, 1)):
                    V.tensor_scalar(tl[:, sl], tl[:, sl], 0.0,
                                    clip[:, cc:cc + 1], Alu.max, Alu.min)
                V.tensor_tensor(aw[:, sl], x2o[:, sl], x1o[:, sl], Alu.subtract)
                V.tensor_scalar_add(aw[:, sl], aw[:, sl], 1.0)
                V.tensor_tensor(area[:, sl], y2o[:, sl], y1o[:, sl], Alu.subtract)
                V.tensor_scalar_add(area[:, sl], area[:, sl], 1.0)
                V.tensor_tensor(area[:, sl], area[:, sl], aw[:, sl], Alu.mult)

            decode_wave(slice(0, KN))

            # ---- stage 4: suppression matrix M[p, i, j] = IoU(i,j) > thr ----
            def iview(t):
                return t[:, :, None].broadcast_to([P, KN, KN])

            def jview(t):
                return t[:, None, :].broadcast_to([P, KN, KN])

            def big():
                _tagn[0] += 1
                tl = pool.tile([P, KN * KN], F32, tag=f"big{_tagn[0]}", name=f"big{_tagn[0]}")
                return tl, tl[:].rearrange("p (i j) -> p i j", i=KN)

            M, Mv = big()
            w_t, w_v = big()
            xx1, xx1v = big()
            V.tensor_tensor(xx1v, iview(x1o), jview(x1o), Alu.max)
            V.tensor_tensor(Mv, iview(x2o), jview(x2o), Alu.min)
            V.tensor_tensor(w_v, Mv, xx1v, Alu.subtract)
            V.tensor_scalar(w_t[:], w_t[:], 1.0, 0.0, Alu.add, Alu.max)
            V.tensor_tensor(xx1v, iview(y1o), jview(y1o), Alu.max)
            V.tensor_tensor(Mv, iview(y2o), jview(y2o), Alu.min)
            V.tensor_tensor(xx1v, Mv, xx1v, Alu.subtract)
            V.tensor_scalar(xx1[:], xx1[:], 1.0, 0.0, Alu.add, Alu.max)
            inter = w_t
            V.tensor_tensor(inter[:], w_t[:], xx1[:], Alu.mult)   # inter
            # IoU > t  <=>  inter > (t/(1+t))*(Ai+Aj); margin |IoU-0.7| on
            # this data is 2e-5 >> the ~1e-7 fp discrepancy of the rewrite
            V.tensor_scalar_mul(area[:], area[:], NMS_THR / (1.0 + NMS_THR))
            V.tensor_tensor(Mv, jview(area), iview(area), Alu.add)
            V.tensor_tensor(M[:], inter[:], M[:], Alu.is_gt)       # M flags

            # ---- stage 5: NMS via fixpoint iteration ----
            # M[p, a, b] = (IoU(a,b) > thr) & (b < a): k_{t+1}[a] =
            # valid[a] & ~any_b(M[a,b] & k_t[b]). The greedy keep set is the
            # unique fixpoint; 2 iterations cover suppression-chain depth 1
            # (measured depth on this data: 1) and the k4==k3 convergence
            # residual ships to the host, which falls back to an exact
            # reference recompute if it is ever nonzero.
            TRI = pool.tile([P, KN * KN], F32, tag="TRI")
            nc.sync.dma_start(TRI[:], bass.AP(tri_in, 0, [[0, P], [1, KN * KN]]))
            V.tensor_tensor(M[:], M[:], TRI[:], Alu.mult)
            valid = pool.tile([P, KN], F32, tag="valid")
            V.tensor_single_scalar(valid[:], vals[:, :KN], SCORE_THR, Alu.is_gt)
            ka = pool.tile([P, KN], F32, tag="ka")
            kb = pool.tile([P, KN], F32, tag="kb")
            supp = pool.tile([P, KN], F32, tag="supp")
            TMP, TMPv = big()
            k_prev, k_cur = None, valid
            for it in range(2):
                k_next = ka if it % 2 == 0 else kb
                V.tensor_tensor(
                    TMPv, Mv,
                    k_cur[:, None, :].broadcast_to([P, KN, KN]), Alu.mult)
                V.tensor_reduce(supp[:], TMPv, mybir.AxisListType.X, Alu.max)
                V.scalar_tensor_tensor(k_next[:], supp[:], 0.0, valid[:],
                                       Alu.is_equal, Alu.mult)
                k_prev, k_cur = k_cur, k_next
            dtile = pool.tile([P, KN], F32, tag="dtile")
            V.tensor_tensor(dtile[:], k_cur[:], k_prev[:], Alu.not_equal)
            dsum = pool.tile([P, 1], F32, tag="dsum")
            V.tensor_reduce(dsum[:], dtile[:], mybir.AxisListType.X, Alu.add)
            nc.sync.dma_start(ometa[:, 32:33], dsum[:])

            # ---- stage 6: masked scores + outputs ----
            good = k_cur
            pen = pool.tile([P, KN], F32, tag="pen")
            V.tensor_scalar(pen[:], good[:], -NEG, NEG, Alu.mult, Alu.add)  # 0 kept, NEG else
            V.tensor_tensor(good[:], good[:], vals[:, :KN], Alu.mult)
            V.tensor_tensor(good[:], good[:], pen[:], Alu.add)
            nc.sync.dma_start(omask[:], good[:])
            for kk, tl in enumerate((x1o, y1o, x2o, y2o)):
                nc.sync.dma_start(oboxes[kk], tl[:])
    if split_waits:
        _split_sync_waits(nc)
    return nc


# ---------------------------------------------------------------- host side

def _prep_core(all_rois, all_box_deltas, all_cls_scores, im_info, core):
    b, h = core // 2, core % 2
    sc = np.full((P, NPAD), NEG, np.float32)
    sc[:, :N] = all_cls_scores.reshape(B, N, C)[b, :, 1 + 40 * h:41 + 40 * h].T
    planes = np.zeros((NT, 8, 16, N), np.float32)
    planes[:, :, 0:4, :] = all_rois[b].T[None, None]
    dsl = all_box_deltas.reshape(B, N, C * 4)[b][:, 4 + 160 * h:164 + 160 * h]
    planes[:, :, 4:8, :] = dsl.T.reshape(NT, 8, 4, N)
    clip = np.empty((P, 2), np.float32)
    clip[:, 0] = np.float32(im_info[b, 1]) - np.float32(1.0)
    clip[:, 1] = np.float32(im_info[b, 0]) - np.float32(1.0)
    a = np.arange(KN)
    tri = (a[None, :] < a[:, None]).astype(np.float32).reshape(-1)
    return {"sc_in": np.ascontiguousarray(sc),
            "planes_in": np.ascontiguousarray(planes.reshape(NT, 128, N)),
            "clip_in": clip, "tri_in": tri}


def _merge(results):
    out_boxes = np.zeros((B, MAX_DET, 4), np.float32)
    out_scores = np.zeros((B, MAX_DET), np.float32)
    out_classes = np.zeros((B, MAX_DET), np.int32)
    ok_all = True
    for b in range(B):
        r0, r1 = results[2 * b], results[2 * b + 1]
        masked = np.concatenate([r0["omask"], r1["omask"]], axis=0)      # [80, K]
        boxes = np.concatenate([r0["oboxes"], r1["oboxes"]], axis=1)     # [4, 80, K]
        cert = max(float(r0["ometa"][:, KN - 1].max()), float(r1["ometa"][:, KN - 1].max()))
        if float(r0["ometa"][:, 32].sum()) != 0.0 or float(r1["ometa"][:, 32].sum()) != 0.0:
            ok_all = False
            break
        flat = masked.reshape(-1)
        order = np.argsort(-flat, kind="stable")[:MAX_DET]
        ssel = flat[order]
        if not (ssel[-1] > cert and ssel[-1] > -1.0e29):
            ok_all = False
            break
        cm_sel = order // KN
        out_scores[b] = ssel
        out_classes[b] = (cm_sel + 1).astype(np.int32)
        bt = boxes.reshape(4, -1)
        out_boxes[b] = bt[:, order].T
    if not ok_all:
        return None
    out_batch = np.repeat(np.arange(B, dtype=np.int32), MAX_DET)
    return (out_boxes.reshape(-1, 4), out_classes.reshape(-1),
            out_scores.reshape(-1), out_batch)


def kernel(all_rois, all_box_deltas, all_cls_scores, im_info, _sim=False, _trace=False):
    global _CACHED_NC
    all_rois = np.asarray(all_rois, np.float32)
    all_box_deltas = np.asarray(all_box_deltas, np.float32)
    all_cls_scores = np.asarray(all_cls_scores, np.float32)
    im_info = np.asarray(im_info, np.float32)

    if _sim:
        nc = build_program(split_waits=False)
    else:
        if _CACHED_NC is None:
            _CACHED_NC = build_program()
        nc = _CACHED_NC
    in_maps = [_prep_core(all_rois, all_box_deltas, all_cls_scores, im_info, c)
               for c in range(8)]

    if _sim:
        from concourse import bass_interp
        results = []
        for c in range(8):
            sim = bass_interp.CoreSim(nc)
            for k, v in in_maps[c].items():
                sim.tensor(k)[:] = v
            sim.simulate()
            results.append({k: np.array(sim.tensor(k))
                            for k in ("oboxes", "omask", "ometa")})
        kr = None
    else:
        from concourse.bass_utils import run_bass_kernel_spmd
        kr = run_bass_kernel_spmd(nc, in_maps, list(range(8)), trace=_trace)
        results = kr.results

    merged = _merge(results)
    if merged is None:
        merged = _numpy_reference_fallback(all_rois, all_box_deltas,
                                           all_cls_scores, im_info)
    if _trace:
        return merged, kr
    return merged


# ---------------- exact numpy fallback (certification guard; never hit on
# ---------------- well-behaved inputs, kept for unconditional correctness)

def _numpy_reference_fallback(all_rois, all_box_deltas, all_cls_scores, im_info):
    deltas = all_box_deltas.reshape(B, N, C, 4)
    scores = all_cls_scores.reshape(B, N, C)
    sc = np.moveaxis(scores[:, :, 1:], 1, 2)
    valid = sc > SCORE_THR
    order = np.argsort(-np.where(valid, sc, -np.inf), axis=-1, kind="stable")
    out_boxes = np.zeros((B, MAX_DET, 4), np.float32)
    out_scores = np.zeros((B, MAX_DET), np.float32)
    out_classes = np.zeros((B, MAX_DET), np.int32)
    for b in range(B):
        Wm = np.float32(im_info[b, 1]) - 1.0
        Hm = np.float32(im_info[b, 0]) - 1.0
        cand_sc = np.full((Cm, N), -np.inf, np.float32)
        cand_bx = np.zeros((Cm, N, 4), np.float32)
        for cm in range(Cm):
            o = order[b, cm]
            r = all_rois[b, o]
            d = deltas[b, o, cm + 1]
            w = r[:, 2] - r[:, 0] + 1.0
            h = r[:, 3] - r[:, 1] + 1.0
            cx = r[:, 0] + 0.5 * w
            cy = r[:, 1] + 0.5 * h
            pcx = d[:, 0] / 10.0 * w + cx
            pcy = d[:, 1] / 10.0 * h + cy
            pw = np.exp(np.minimum(d[:, 2] / 5.0, np.float32(MAX_LOG_WH))) * w
            ph = np.exp(np.minimum(d[:, 3] / 5.0, np.float32(MAX_LOG_WH))) * h
            x1 = np.clip(pcx - 0.5 * pw, 0, Wm)
            y1 = np.clip(pcy - 0.5 * ph, 0, Hm)
            x2 = np.clip(pcx + 0.5 * pw - 1.0, 0, Wm)
            y2 = np.clip(pcy + 0.5 * ph - 1.0, 0, Hm)
            area = (x2 - x1 + 1.0) * (y2 - y1 + 1.0)
            s = sc[b, cm, o]
            keep = s > SCORE_THR
            for i in range(N):
                if not keep[i]:
                    continue
                xx1 = np.maximum(x1, x1[i]); yy1 = np.maximum(y1, y1[i])
                xx2 = np.minimum(x2, x2[i]); yy2 = np.minimum(y2, y2[i])
                inter = (np.maximum(xx2 - xx1 + 1.0, 0.0)
                         * np.maximum(yy2 - yy1 + 1.0, 0.0))
                iou = inter / (area + area[i] - inter)
                supp = (iou > NMS_THR) & (np.arange(N) > i) & keep
                keep &= ~supp
            cand_sc[cm, keep] = s[keep]
            cand_bx[cm] = np.stack([x1, y1, x2, y2], -1)
        flat = cand_sc.reshape(-1)
        o = np.argsort(-flat, kind="stable")[:MAX_DET]
        okm = np.isfinite(flat[o])
        out_scores[b] = np.where(okm, flat[o], 0.0)
        out_boxes[b] = np.where(okm[:, None], cand_bx.reshape(-1, 4)[o], 0.0)
        out_classes[b] = np.where(okm, o // N + 1, 0).astype(np.int32)
    out_batch = np.repeat(np.arange(B, dtype=np.int32), MAX_DET)
    return (out_boxes.reshape(-1, 4), out_classes.reshape(-1),
            out_scores.reshape(-1), out_batch)


# revision 44
# speedup vs baseline: 1.1793x; 1.0109x over previous
"""Trainium2 Bass kernel for nn_DetectionOutput (decode + per-class NMS + top-k).

Sharding: 8 cores = 4 images x 2 class-halves. Core c handles image b=c//2,
classes cm in [40h, 40h+40) where h=c%2 (cm = class-1, i.e. background dropped).

Algorithm (exact, certified): with uniform scores the per-image top-100 cutoff
is ~0.999 while the 25th-best score of any class is <=0.993, so only the top
KN=24 boxes per class can reach the output. Greedy-NMS keep of a sorted prefix
depends only on that prefix, so each core:
  1. 4 max8 rounds per class -> top-24 scores+indices, rank-24 cert value
     (DVE max8/find_index8/match_replace; ties resolve index-ascending,
     matching jnp stable argsort)
  2. per-round GPSIMD indirect_copy gathers of roi+delta planes for the 8
     freshly selected boxes, hidden under the next round's DVE work
  3. decode + clip boxes                            (DVE + one ACT exp)
  4. 24x24 IoU>0.7 strict-lower suppression matrix  (DVE broadcast ops)
  5. NMS keep = fixpoint of k <- valid & ~(M k), 3 iterations (covers
     suppression-chain depth 2; measured depth on this data is 1), with the
     k3==k2 residual shipped to the host
  6. emits kept-masked scores, boxes, cert + convergence meta
Host merges the two half-image candidate sets per image with the reference
tie-break (score desc, candidate index asc), certifies the prefix bound
(tau_100 > max cert, margin ~0.007 on uniform scores) and the fixpoint
residual; an exact numpy fallback runs if either check ever fails, so the
kernel is exact for any input.
"""
import sys

sys.path.insert(0, "/opt/trn_rl_repo")

import numpy as np
import concourse.bass as bass
import concourse.mybir as mybir
from concourse.tile import TileContext

F32 = mybir.dt.float32
U32 = mybir.dt.uint32
U16 = mybir.dt.uint16
Alu = mybir.AluOpType
Act = mybir.ActivationFunctionType

B, N, C = 4, 2000, 81
Cm = C - 1
P = 40            # class-problems per core
NPAD = N  # no pad: max8 family accepts any free size 8..16384
K = 32            # scores extracted per class (4 max8 rounds)
KN = 24           # NMS prefix = gathered slots; cert score = rank KN
NROUND = KN // 8      # 3 rounds; cert = vals[:, KN-1] (upper-bounds all unexamined)
NT = 5            # gather tile-groups of 8 problems
GW = NT * KN      # gather scratch cols
MAX_DET = 100
SCORE_THR = 0.01
NMS_THR = 0.7
MAX_LOG_WH = float(np.log(1000.0 / 16.0))
NEG = -1.0e30

_CACHED_NC = None


def _patch_tile_tail_drain():
    """This walrus build rejects CTRL instructions carrying >2 sync waits
    (NCC_INLA001 'Too many sync wait commands' on the Tile tail drain).
    Emit sync-engine NOPs before the drain and spread the waits out, one
    per instruction."""
    import concourse.tile as tile_mod
    from concourse.vector_clock import ScopedClock

    if getattr(tile_mod.TileContext, "_tail_drain_patched", False):
        return

    def _drain_and_barrier(self, tick_clock, wait_clock):
        nc = self.nc
        nops = [nc.sync.nop(nofuse=True) for _ in range(30)]
        drain_inst = nc.sync.drain()
        wait_clock.add_sem_waits(
            drain_inst.ins, ScopedClock({None: tick_clock.global_clock})
        )
        waits = list(drain_inst.ins.sync_info.on_wait or [])
        if len(waits) > 1:
            assert len(waits) <= len(nops) + 1
            drain_inst.ins.sync_info.on_wait = [waits[0]]
            for w, nop in zip(waits[1:], nops):
                nop.ins.sync_info = mybir.SyncInfo(on_wait=[w], on_update=[])
        nc.all_engine_barrier()
        assert self.sems is not None
        popped = nc._tile_sem_poison_stack.pop()
        assert popped is self._sem_poison
        nc.clear_and_free_semaphores(list(self.sems.allocated().values()))
        nc.all_engine_barrier()

    tile_mod.TileContext._drain_and_barrier = _drain_and_barrier
    tile_mod.TileContext._tail_drain_patched = True


def _split_sync_waits(nc, max_waits=1):
    """Walrus codegen in this container rejects instructions carrying more
    than 1-2 sync waits (class-dependent). Cap every instruction at
    `max_waits` by hoisting the excess onto same-engine NoOps inserted
    immediately before it (engine blocks at the same program point, so
    semantics and Tile's schedule-order guarantees are preserved)."""
    import bass_rust
    ctr = 0
    for f in nc.m.functions:
        for bb in f.blocks:
            ins_list = bb.instructions
            new = []
            for inst in ins_list:
                si = inst.sync_info
                waits = list(si.on_wait) if si and si.on_wait else []
                if len(waits) > max_waits:
                    for w in waits[max_waits:]:
                        ctr += 1
                        nop = bass_rust.InstNoOp(name=f"WSPLIT-{ctr}")
                        nop.engine = inst.engine
                        nop.sync_info = mybir.SyncInfo(on_wait=[w], on_update=[])
                        new.append(nop)
                    inst.sync_info = mybir.SyncInfo(
                        on_wait=waits[:max_waits],
                        on_update=list(si.on_update) if si.on_update else [])
                new.append(inst)
            ins_list[:] = new
    return ctr


def build_program(split_waits=True):
    _patch_tile_tail_drain()
    nc = bass.Bass()
    sc_in = nc.dram_tensor("sc_in", [P, NPAD], F32, kind="ExternalInput")
    planes_in = nc.dram_tensor("planes_in", [NT, 128, N], F32, kind="ExternalInput")
    clip_in = nc.dram_tensor("clip_in", [P, 2], F32, kind="ExternalInput")  # [Wm, Hm]
    oboxes = nc.dram_tensor("oboxes", [4, P, KN], F32, kind="ExternalOutput")
    omask = nc.dram_tensor("omask", [P, KN], F32, kind="ExternalOutput")
    ometa = nc.dram_tensor("ometa", [P, 40], F32, kind="ExternalOutput")
    tri_in = nc.dram_tensor("tri_in", [KN * KN], F32, kind="ExternalInput")
    scr_gat = nc.dram_tensor("scr_gat", [128 * GW], F32)

    with TileContext(nc) as tc:
        with (
            tc.tile_pool(name="pool", bufs=1) as pool,
            tc.tile_pool(name="plpool", bufs=1) as plpool,
        ):
            sc = pool.tile([P, NPAD], F32)
            nc.sync.dma_start(sc[:], sc_in[:])
            clip = pool.tile([P, 2], F32)
            nc.sync.dma_start(clip[:], clip_in[:])

            # plane tiles up-front; their DMAs overlap the topk rounds
            pls = []
            for t in range(NT):
                pl = plpool.tile([128, N], F32, tag=f"pl{t}", name=f"pl{t}")
                nc.sync.dma_start(pl[:], planes_in[t])
                pls.append(pl)

            # ---- stage 1+2: topk rounds with per-round gather ----
            # idxr row p holds round r's 8 indices at cols 16r..16r+8 (u16);
            # iwt_rt[16g+j, 0] = idxr[8t+g, 16r+j] so indirect_copy's "(s p)"
            # unwrap yields slots 8r..8r+7 in order. Gathers for round r run
            # on GPSIMD under round r+1's DVE work.
            vals = pool.tile([P, NROUND * 8], F32)
            idxr = pool.tile([P, 16 * NROUND], U16)
            nc.vector.memset(idxr[:], 0)
            gat = pool.tile([128, NT * KN], F32)
            irap = idxr[:]
            pk = []
            for k in range(8):
                pkt = pool.tile([P, KN], F32, tag=f"pk{k}", name=f"pk{k}")
                pk.append(pkt)
            for r in range(NROUND):
                s8 = slice(r * 8, (r + 1) * 8)
                nc.vector.max(vals[:, s8], sc[:])
                if r * 8 >= KN:
                    continue    # cert-only round: no indices, no gather
                nc.vector.max_index(idxr[:, 16 * r:16 * r + 8], vals[:, s8], sc[:])
                if r + 1 < NROUND:
                    nc.vector.match_replace(sc[:], vals[:, s8], sc[:], NEG)
                for t in range(NT):
                    iwt = plpool.tile([128, 1], U16, tag=f"iw{t}_{r}",
                                      name=f"iw{t}_{r}")
                    nc.sync.dma_start(
                        iwt[:], bass.AP(irap.tensor,
                                        irap.offset + 8 * t * 16 * NROUND + 16 * r,
                                        [[16 * NROUND, 8], [1, 16], [1, 1]]))
                    nc.gpsimd.indirect_copy(
                        gat[:, t * KN + 8 * r:t * KN + 8 * r + 8].rearrange(
                            "p (i one) -> p i one", one=1),
                        pls[t][:], iwt[:], True)
                # bounce this round's gathered columns to DRAM for un-interleave
                gap = gat[:]
                nc.sync.dma_start(
                    bass.AP(scr_gat, 8 * r, [[GW, 128], [KN, NT], [1, 8]]),
                    bass.AP(gap.tensor, gap.offset + 8 * r,
                            [[GW, 128], [KN, NT], [1, 8]]))
            nc.sync.dma_start(ometa[:, 0:NROUND * 8], vals[:])
            for k in range(8):
                nc.sync.dma_start(
                    pk[k][:], bass.AP(scr_gat, k * GW, [[KN, NT], [16 * GW, 8], [1, KN]]))
            x1r, y1r, x2r, y2r, dx, dy, dw, dh = pk

            _tagn = [0]

            def tile():
                _tagn[0] += 1
                return pool.tile([P, KN], F32, tag=f"dec{_tagn[0]}", name=f"dec{_tagn[0]}")

            V = nc.vector
            # ---- stage 3: decode (mirrors reference fp op order) ----
            w = pool.tile([P, KN], F32, tag="w")
            h = pool.tile([P, KN], F32, tag="h")
            cx, cy, pcx, pcy, pw, ph, area = (tile() for _ in range(7))
            ew = pool.tile([P, 2 * KN], F32, tag="ew")
            aw = pool.tile([P, KN], F32, tag="aw")
            x1o, y1o, x2o, y2o = x1r, y1r, x2r, y2r

            def decode_wave(sl):
                s2 = slice(KN + sl.start, KN + sl.stop)
                V.tensor_tensor(w[:, sl], x2r[:, sl], x1r[:, sl], Alu.subtract)
                V.tensor_scalar_add(w[:, sl], w[:, sl], 1.0)
                V.tensor_tensor(h[:, sl], y2r[:, sl], y1r[:, sl], Alu.subtract)
                V.tensor_scalar_add(h[:, sl], h[:, sl], 1.0)
                V.tensor_scalar_mul(cx[:, sl], w[:, sl], 0.5)
                V.tensor_tensor(cx[:, sl], x1r[:, sl], cx[:, sl], Alu.add)
                V.tensor_scalar_mul(cy[:, sl], h[:, sl], 0.5)
                V.tensor_tensor(cy[:, sl], y1r[:, sl], cy[:, sl], Alu.add)
                V.tensor_scalar_mul(pcx[:, sl], dx[:, sl], 0.1)
                V.tensor_tensor(pcx[:, sl], pcx[:, sl], w[:, sl], Alu.mult)
                V.tensor_tensor(pcx[:, sl], pcx[:, sl], cx[:, sl], Alu.add)
                V.tensor_scalar_mul(pcy[:, sl], dy[:, sl], 0.1)
                V.tensor_tensor(pcy[:, sl], pcy[:, sl], h[:, sl], Alu.mult)
                V.tensor_tensor(pcy[:, sl], pcy[:, sl], cy[:, sl], Alu.add)
                V.tensor_scalar(ew[:, sl], dw[:, sl], 0.2, MAX_LOG_WH,
                                Alu.mult, Alu.min)
                V.tensor_scalar(ew[:, s2], dh[:, sl], 0.2, MAX_LOG_WH,
                                Alu.mult, Alu.min)
                nc.scalar.activation(ew[:, sl], ew[:, sl], Act.Exp)
                nc.scalar.activation(ew[:, s2], ew[:, s2], Act.Exp)
                V.tensor_tensor(pw[:, sl], ew[:, sl], w[:, sl], Alu.mult)
                V.tensor_tensor(ph[:, sl], ew[:, s2], h[:, sl], Alu.mult)
                V.tensor_scalar_mul(pw[:, sl], pw[:, sl], 0.5)
                V.tensor_tensor(x1o[:, sl], pcx[:, sl], pw[:, sl], Alu.subtract)
                V.tensor_tensor(x2o[:, sl], pcx[:, sl], pw[:, sl], Alu.add)
                V.tensor_scalar_add(x2o[:, sl], x2o[:, sl], -1.0)
                V.tensor_scalar_mul(ph[:, sl], ph[:, sl], 0.5)
                V.tensor_tensor(y1o[:, sl], pcy[:, sl], ph[:, sl], Alu.subtract)
                V.tensor_tensor(y2o[:, sl], pcy[:, sl], ph[:, sl], Alu.add)
                V.tensor_scalar_add(y2o[:, sl], y2o[:, sl], -1.0)
                for tl, cc in ((x1o, 0), (x2o, 0), (y1o, 1), (y2o, 1)):
                    V.tensor_scalar(tl[:, sl], tl[:, sl], 0.0,
                                    clip[:, cc:cc + 1], Alu.max, Alu.min)
                V.tensor_tensor(aw[:, sl], x2o[:, sl], x1o[:, sl], Alu.subtract)
                V.tensor_scalar_add(aw[:, sl], aw[:, sl], 1.0)
                V.tensor_tensor(area[:, sl], y2o[:, sl], y1o[:, sl], Alu.subtract)
                V.tensor_scalar_add(area[:, sl], area[:, sl], 1.0)
                V.tensor_tensor(area[:, sl], area[:, sl], aw[:, sl], Alu.mult)

            decode_wave(slice(0, KN))

            # ---- stage 4: suppression matrix M[p, i, j] = IoU(i,j) > thr ----
            def iview(t):
                return t[:, :, None].broadcast_to([P, KN, KN])

            def jview(t):
                return t[:, None, :].broadcast_to([P, KN, KN])

            def big():
                _tagn[0] += 1
                tl = pool.tile([P, KN * KN], F32, tag=f"big{_tagn[0]}", name=f"big{_tagn[0]}")
                return tl, tl[:].rearrange("p (i j) -> p i j", i=KN)

            M, Mv = big()
            w_t, w_v = big()
            xx1, xx1v = big()
            V.tensor_tensor(xx1v, iview(x1o), jview(x1o), Alu.max)
            V.tensor_tensor(Mv, iview(x2o), jview(x2o), Alu.min)
            V.tensor_tensor(w_v, Mv, xx1v, Alu.subtract)
            V.tensor_scalar(w_t[:], w_t[:], 1.0, 0.0, Alu.add, Alu.max)
            V.tensor_tensor(xx1v, iview(y1o), jview(y1o), Alu.max)
            V.tensor_tensor(Mv, iview(y2o), jview(y2o), Alu.min)
            V.tensor_tensor(xx1v, Mv, xx1v, Alu.subtract)
            V.tensor_scalar(xx1[:], xx1[:], 1.0, 0.0, Alu.add, Alu.max)
            inter = w_t
            V.tensor_tensor(inter[:], w_t[:], xx1[:], Alu.mult)   # inter
            # IoU > t  <=>  inter > (t/(1+t))*(Ai+Aj); margin |IoU-0.7| on
            # this data is 2e-5 >> the ~1e-7 fp discrepancy of the rewrite
            V.tensor_scalar_mul(area[:], area[:], NMS_THR / (1.0 + NMS_THR))
            V.tensor_tensor(Mv, jview(area), iview(area), Alu.add)
            V.tensor_tensor(M[:], inter[:], M[:], Alu.is_gt)       # M flags

            # ---- stage 5: NMS via fixpoint iteration ----
            # M[p, a, b] = (IoU(a,b) > thr) & (b < a): k_{t+1}[a] =
            # valid[a] & ~any_b(M[a,b] & k_t[b]). The greedy keep set is the
            # unique fixpoint; 2 iterations cover suppression-chain depth 1
            # (measured depth on this data: 1) and the k4==k3 convergence
            # residual ships to the host, which falls back to an exact
            # reference recompute if it is ever nonzero.
            TRI = pool.tile([P, KN * KN], F32, tag="TRI")
            nc.sync.dma_start(TRI[:], bass.AP(tri_in, 0, [[0, P], [1, KN * KN]]))
            V.tensor_tensor(M[:], M[:], TRI[:], Alu.mult)
            valid = pool.tile([P, KN], F32, tag="valid")
            V.tensor_single_scalar(valid[:], vals[:, :KN], SCORE_THR, Alu.is_gt)
            ka = pool.tile([P, KN], F32, tag="ka")
            kb = pool.tile([P, KN], F32, tag="kb")
            supp = pool.tile([P, KN], F32, tag="supp")
            TMP, TMPv = big()
            k_prev, k_cur = None, valid
            for it in range(2):
                k_next = ka if it % 2 == 0 else kb
                V.tensor_tensor(
                    TMPv, Mv,
                    k_cur[:, None, :].broadcast_to([P, KN, KN]), Alu.mult)
                V.tensor_reduce(supp[:], TMPv, mybir.AxisListType.X, Alu.max)
                V.scalar_tensor_tensor(k_next[:], supp[:], 0.0, valid[:],
                                       Alu.is_equal, Alu.mult)
                k_prev, k_cur = k_cur, k_next
            dtile = pool.tile([P, KN], F32, tag="dtile")
            V.tensor_tensor(dtile[:], k_cur[:], k_prev[:], Alu.not_equal)
            dsum = pool.tile([P, 1], F32, tag="dsum")
            V.tensor_reduce(dsum[:], dtile[:], mybir.AxisListType.X, Alu.add)
            nc.sync.dma_start(ometa[:, 32:33], dsum[:])

            # ---- stage 6: masked scores + outputs ----
            good = k_cur
            pen = pool.tile([P, KN], F32, tag="pen")
            V.tensor_scalar(pen[:], good[:], -NEG, NEG, Alu.mult, Alu.add)  # 0 kept, NEG else
            V.tensor_tensor(good[:], good[:], vals[:, :KN], Alu.mult)
            V.tensor_tensor(good[:], good[:], pen[:], Alu.add)
            nc.sync.dma_start(omask[:], good[:])
            for kk, tl in enumerate((x1o, y1o, x2o, y2o)):
                nc.sync.dma_start(oboxes[kk], tl[:])
    if split_waits:
        _split_sync_waits(nc)
    return nc


# ---------------------------------------------------------------- host side

def _prep_core(all_rois, all_box_deltas, all_cls_scores, im_info, core):
    b, h = core // 2, core % 2
    sc = np.ascontiguousarray(
        all_cls_scores.reshape(B, N, C)[b, :, 1 + 40 * h:41 + 40 * h].T)
    planes = np.zeros((NT, 8, 16, N), np.float32)
    planes[:, :, 0:4, :] = all_rois[b].T[None, None]
    dsl = all_box_deltas.reshape(B, N, C * 4)[b][:, 4 + 160 * h:164 + 160 * h]
    planes[:, :, 4:8, :] = dsl.T.reshape(NT, 8, 4, N)
    clip = np.empty((P, 2), np.float32)
    clip[:, 0] = np.float32(im_info[b, 1]) - np.float32(1.0)
    clip[:, 1] = np.float32(im_info[b, 0]) - np.float32(1.0)
    a = np.arange(KN)
    tri = (a[None, :] < a[:, None]).astype(np.float32).reshape(-1)
    return {"sc_in": sc,
            "planes_in": np.ascontiguousarray(planes.reshape(NT, 128, N)),
            "clip_in": clip, "tri_in": tri}


def _merge(results):
    out_boxes = np.zeros((B, MAX_DET, 4), np.float32)
    out_scores = np.zeros((B, MAX_DET), np.float32)
    out_classes = np.zeros((B, MAX_DET), np.int32)
    ok_all = True
    for b in range(B):
        r0, r1 = results[2 * b], results[2 * b + 1]
        masked = np.concatenate([r0["omask"], r1["omask"]], axis=0)      # [80, K]
        boxes = np.concatenate([r0["oboxes"], r1["oboxes"]], axis=1)     # [4, 80, K]
        cert = max(float(r0["ometa"][:, KN - 1].max()), float(r1["ometa"][:, KN - 1].max()))
        if float(r0["ometa"][:, 32].sum()) != 0.0 or float(r1["ometa"][:, 32].sum()) != 0.0:
            ok_all = False
            break
        flat = masked.reshape(-1)
        order = np.argsort(-flat, kind="stable")[:MAX_DET]
        ssel = flat[order]
        if not (ssel[-1] > cert and ssel[-1] > -1.0e29):
            ok_all = False
            break
        cm_sel = order // KN
        out_scores[b] = ssel
        out_classes[b] = (cm_sel + 1).astype(np.int32)
        bt = boxes.reshape(4, -1)
        out_boxes[b] = bt[:, order].T
    if not ok_all:
        return None
    out_batch = np.repeat(np.arange(B, dtype=np.int32), MAX_DET)
    return (out_boxes.reshape(-1, 4), out_classes.reshape(-1),
            out_scores.reshape(-1), out_batch)


def kernel(all_rois, all_box_deltas, all_cls_scores, im_info, _sim=False, _trace=False):
    global _CACHED_NC
    all_rois = np.asarray(all_rois, np.float32)
    all_box_deltas = np.asarray(all_box_deltas, np.float32)
    all_cls_scores = np.asarray(all_cls_scores, np.float32)
    im_info = np.asarray(im_info, np.float32)

    if _sim:
        nc = build_program(split_waits=False)
    else:
        if _CACHED_NC is None:
            _CACHED_NC = build_program()
        nc = _CACHED_NC
    in_maps = [_prep_core(all_rois, all_box_deltas, all_cls_scores, im_info, c)
               for c in range(8)]

    if _sim:
        from concourse import bass_interp
        results = []
        for c in range(8):
            sim = bass_interp.CoreSim(nc)
            for k, v in in_maps[c].items():
                sim.tensor(k)[:] = v
            sim.simulate()
            results.append({k: np.array(sim.tensor(k))
                            for k in ("oboxes", "omask", "ometa")})
        kr = None
    else:
        from concourse.bass_utils import run_bass_kernel_spmd
        kr = run_bass_kernel_spmd(nc, in_maps, list(range(8)), trace=_trace)
        results = kr.results

    merged = _merge(results)
    if merged is None:
        merged = _numpy_reference_fallback(all_rois, all_box_deltas,
                                           all_cls_scores, im_info)
    if _trace:
        return merged, kr
    return merged


# ---------------- exact numpy fallback (certification guard; never hit on
# ---------------- well-behaved inputs, kept for unconditional correctness)

def _numpy_reference_fallback(all_rois, all_box_deltas, all_cls_scores, im_info):
    deltas = all_box_deltas.reshape(B, N, C, 4)
    scores = all_cls_scores.reshape(B, N, C)
    sc = np.moveaxis(scores[:, :, 1:], 1, 2)
    valid = sc > SCORE_THR
    order = np.argsort(-np.where(valid, sc, -np.inf), axis=-1, kind="stable")
    out_boxes = np.zeros((B, MAX_DET, 4), np.float32)
    out_scores = np.zeros((B, MAX_DET), np.float32)
    out_classes = np.zeros((B, MAX_DET), np.int32)
    for b in range(B):
        Wm = np.float32(im_info[b, 1]) - 1.0
        Hm = np.float32(im_info[b, 0]) - 1.0
        cand_sc = np.full((Cm, N), -np.inf, np.float32)
        cand_bx = np.zeros((Cm, N, 4), np.float32)
        for cm in range(Cm):
            o = order[b, cm]
            r = all_rois[b, o]
            d = deltas[b, o, cm + 1]
            w = r[:, 2] - r[:, 0] + 1.0
            h = r[:, 3] - r[:, 1] + 1.0
            cx = r[:, 0] + 0.5 * w
            cy = r[:, 1] + 0.5 * h
            pcx = d[:, 0] / 10.0 * w + cx
            pcy = d[:, 1] / 10.0 * h + cy
            pw = np.exp(np.minimum(d[:, 2] / 5.0, np.float32(MAX_LOG_WH))) * w
            ph = np.exp(np.minimum(d[:, 3] / 5.0, np.float32(MAX_LOG_WH))) * h
            x1 = np.clip(pcx - 0.5 * pw, 0, Wm)
            y1 = np.clip(pcy - 0.5 * ph, 0, Hm)
            x2 = np.clip(pcx + 0.5 * pw - 1.0, 0, Wm)
            y2 = np.clip(pcy + 0.5 * ph - 1.0, 0, Hm)
            area = (x2 - x1 + 1.0) * (y2 - y1 + 1.0)
            s = sc[b, cm, o]
            keep = s > SCORE_THR
            for i in range(N):
                if not keep[i]:
                    continue
                xx1 = np.maximum(x1, x1[i]); yy1 = np.maximum(y1, y1[i])
                xx2 = np.minimum(x2, x2[i]); yy2 = np.minimum(y2, y2[i])
                inter = (np.maximum(xx2 - xx1 + 1.0, 0.0)
                         * np.maximum(yy2 - yy1 + 1.0, 0.0))
                iou = inter / (area + area[i] - inter)
                supp = (iou > NMS_THR) & (np.arange(N) > i) & keep
                keep &= ~supp
            cand_sc[cm, keep] = s[keep]
            cand_bx[cm] = np.stack([x1, y1, x2, y2], -1)
        flat = cand_sc.reshape(-1)
        o = np.argsort(-flat, kind="stable")[:MAX_DET]
        okm = np.isfinite(flat[o])
        out_scores[b] = np.where(okm, flat[o], 0.0)
        out_boxes[b] = np.where(okm[:, None], cand_bx.reshape(-1, 4)[o], 0.0)
        out_classes[b] = np.where(okm, o // N + 1, 0).astype(np.int32)
    out_batch = np.repeat(np.arange(B, dtype=np.int32), MAX_DET)
    return (out_boxes.reshape(-1, 4), out_classes.reshape(-1),
            out_scores.reshape(-1), out_batch)
